# revision 1
# baseline (speedup 1.0000x reference)
"""MiMo V2 MoE gate (sigmoid routing, grouped top-k) on 8 Trainium2 cores.

Contract: kernel(**inputs) takes the FULL unsharded inputs
(hidden_states [4,4096,4096] f32, weight [256,4096] f32,
e_score_correction_bias [256] f32) and returns (topk_idx int32 [16384,8],
topk_weight f32 [16384,8]) matching reference.py.

Strategy (data-parallel over tokens):
  - 16384 tokens are sharded 2048/core across 8 NeuronCores.
  - Host pre-transposes each x shard to [4096, 2048] so the device can
    feed the PE's contraction (partition) dim directly; weight.T [4096,256]
    and the bias (pre-broadcast to [128,256]) are replicated.
  - Per core: gate GEMM in fp32 (PSUM accumulation over 32 k-chunks),
    sigmoid on ScalarE, then the grouped top-k entirely on VectorE using
    the DVE sort8 primitives (max / max_index / match_replace). The
    topk weights (scores at the selected experts, ordered by biased
    score rank) are recovered without any gather via an 8x8 index-match
    between the two sort orders.
"""

import sys

if "/opt/trn_rl_repo" not in sys.path:
    sys.path.insert(0, "/opt/trn_rl_repo")

import numpy as np

import concourse.bass as bass
import concourse.mybir as mybir
import concourse.tile as tile
from concourse.tile_rust import add_dep_helper, annotate_deps

P = 128
H = 4096
E = 256
N_CORES = 8
T_FULL = 16384
T_CORE = T_FULL // N_CORES  # 2048
KC = H // P                 # 32 contraction chunks
TOK_TILES = T_CORE // P     # 16 token tiles per core
N_GROUP = 8
EG = E // N_GROUP           # 32 experts per group
TOPK_GROUP = 4
TOP_K = 8
ROUTED_SCALING = 2.5
NEG = -1e30

F32 = mybir.dt.float32
U32 = mybir.dt.uint32
AF = mybir.ActivationFunctionType
OP = mybir.AluOpType

# dtype used for the matmul operand tiles (float32 = exact 4-pass;
# float32r = relaxed-precision full-speed variant)
MM_DT = F32


def _reserve(nc, eng, X, n, prev=None):
    """Emit n plain nops on X's engine, ordered after `prev` (a
    BassInstruction or None) and before X. They act as spare 1-wait
    carriers for _legalize_waits (every TPB instruction has exactly one
    HW wait slot; Tile can assign several waits to one instruction,
    which walrus then rejects)."""
    last = prev.ins if prev is not None else None
    for _ in range(n):
        nop = eng.nop(nofuse=True)
        if last is not None:
            add_dep_helper(nop.ins, last, sync=False,
                           reason="chain reserve nop after predecessor")
        add_dep_helper(X.ins, nop.ins, sync=False,
                       reason="reserve nop precedes its instruction")
        last = nop.ins


def _legalize_waits(nc, report=None):
    """Every TPB instruction has ONE hardware wait slot; Tile can assign
    several on_wait entries to an instruction, which walrus rejects
    ("Too many sync wait commands"). Fix in two ways, per engine stream
    (scheduled order):
      1. value-floor dedup: drop waits already implied by an earlier wait
         on the same semaphore in this stream (monotonic sems).
      2. excess-wait hoisting: move extra waits onto the nearest earlier
         wait-free instruction, scanning only across instructions with no
         on_update (pure nops) -- crossing an updater could reorder a
         producer chain and deadlock; this rule keeps placements provably
         safe. _reserve() plants such nops next to risky instructions.
    Drains are skipped (they encode multi-sem waits natively)."""
    stop_types = (
        mybir.InstDrain,
        mybir.InstEventSemaphore,
        mybir.InstCall,
    )
    leftover = []
    if True:
        # The kernel CFG is linear (main block -> end block), so per-engine
        # program order is the block-order concatenation. Crossing an
        # unconditional branch just means waiting before the jump.
        streams = {}
        nonmono = set()  # sems that ever decrease (barrier sems): no
                         # floor-dedup and no relocation for their waits
        for blk in nc.m.functions[0].blocks:
            for inst in blk.instructions:
                streams.setdefault(str(inst.engine), []).append(inst)
                si = inst.sync_info
                for u in (si.on_update if si and si.on_update else []):
                    if str(u.update_mode) not in ('sem-inc', 'sem-add-imm'):
                        nonmono.add(u.id)
        for stream in streams.values():
            floor = {}
            for i, X in enumerate(stream):
                si = X.sync_info
                if si is None or not si.on_wait:
                    continue
                mode_ok = lambda w: (str(w.wait_mode) == 'sem-ge-imm'
                                     and w.id not in nonmono)
                waits = []
                for w in si.on_wait:
                    if (mode_ok(w) and w.id in floor
                            and floor[w.id] >= w.wait_value):
                        continue  # already implied earlier in this stream
                    waits.append(w)
                moved = []
                if len(waits) > 1:
                    # only sem-ge waits are relocatable; sem-sub barrier
                    # ops must stay exactly where Tile put them
                    fixed = [w for w in waits if not mode_ok(w)]
                    movable = [w for w in waits if mode_ok(w)]
                    keep = fixed + movable[:max(0, 1 - len(fixed))]
                    maybe_move = movable[max(0, 1 - len(fixed)):]
                    for w in maybe_move:
                        placed = False
                        for k in range(i - 1, -1, -1):
                            C = stream[k]
                            if isinstance(C, stop_types):
                                break
                            csi = C.sync_info
                            if csi and csi.on_update:
                                break  # never cross a semaphore producer
                            cw = list(csi.on_wait) if csi and csi.on_wait else []
                            if cw or isinstance(
                                    C, mybir.InstUnconditionalBranch):
                                continue  # occupied/branch; keep scanning
                                          # (same-sequencer waits commute)
                            C.sync_info = mybir.SyncInfo(on_wait=[w],
                                                         on_update=[])
                            placed = True
                            break
                        if placed:
                            moved.append(w)
                        else:
                            keep.append(w)
                    waits = keep
                for w in list(waits) + moved:
                    if mode_ok(w):
                        floor[w.id] = max(floor.get(w.id, 0), w.wait_value)
                X.sync_info = mybir.SyncInfo(
                    on_wait=waits,
                    on_update=list(si.on_update) if si.on_update else [])
                if len(waits) > 1:
                    leftover.append((X.name, str(X.engine),
                                     type(X).__name__, len(waits)))
    # The PE gate ENGINE_NOPs carry AP operands purely for Tile dep
    # tracking; walrus's engine check rejects a nop with operands, so
    # strip them now (tile.py does the same for InstNoOp instructions).
    for blk in nc.m.functions[0].blocks:
        for inst in blk.instructions:
            if (isinstance(inst, mybir.InstISA) and (inst.ins or inst.outs)
                    and inst.op_name == 'ENGINE_NOP'):
                inst.ins = []
                inst.outs = []

    if report is not None:
        report.extend(leftover)
    elif leftover:
        raise RuntimeError(f"wait legalization failed for: {leftover}")


def build_nc(mm_dt=MM_DT):
    nc = bass.Bass()

    xT = nc.dram_tensor("xT", [H, T_CORE], F32, kind="ExternalInput")
    wT = nc.dram_tensor("wT", [H, E], F32, kind="ExternalInput")
    biasb = nc.dram_tensor("biasb", [P, E], F32, kind="ExternalInput")
    idx_out = nc.dram_tensor("idx_out", [T_CORE, TOP_K], U32, kind="ExternalOutput")
    w_out = nc.dram_tensor("w_out", [T_CORE, TOP_K], F32, kind="ExternalOutput")

    xT3 = xT.ap().rearrange("(c p) t -> p c t", p=P)      # [128, 32, 2048]
    wT3 = wT.ap().rearrange("(c p) e -> p c e", p=P)      # [128, 32, 256]
    idx3 = idx_out.ap().rearrange("(j p) k -> p j k", p=P)  # [128, 16, 8]
    w3 = w_out.ap().rearrange("(j p) k -> p j k", p=P)

    with tile.TileContext(nc) as tc:
        with (
            tc.tile_pool(name="const", bufs=1) as cpool,
            # bufs=8 so an xt slot's previous DMA sits 8 queue-round-robin
            # steps back -> same HWDGE queue -> WAW covered by queue FIFO,
            # leaving each xt DMA a single (PE slot-release) wait.
            tc.tile_pool(name="xin", bufs=8) as xpool,
            tc.tile_pool(name="psum", bufs=2, space="PSUM") as pspool,
            tc.tile_pool(name="work", bufs=2) as wpool,
        ):
            wsb = cpool.tile([P, KC, E], mm_dt)
            nc.sync.dma_start(wsb[:], wT3)
            bsb = cpool.tile([P, E], F32)
            nc.sync.dma_start(bsb[:], biasb.ap())
            # Persistent per-core output accumulators: no slot reuse, so
            # the DVE producers of idx/w never wait on output DMAs.
            idx_all = cpool.tile([P, TOK_TILES, TOP_K], U32)
            w_all = cpool.tile([P, TOK_TILES, TOP_K], F32)

            prev_sig = None
            prev_mm = None
            prev_dma = None
            last_wout = None
            for j in range(TOK_TILES):
                # ---- gate GEMM: logits[128 tok, 256 exp] ----
                xt = xpool.tile([P, KC, P], mm_dt, tag="xt")
                xt_dma = nc.sync.dma_start(xt[:], xT3[:, :, j * P:(j + 1) * P])
                _reserve(nc, nc.sync, xt_dma, 3, prev=prev_dma)
                prev_dma = xt_dma
                ps = pspool.tile([P, E], F32, tag="ps")
                # The fused fp32 matmul (self-loading LDWEIGHTS) only has
                # budget for ONE semaphore wait in walrus codegen, but the
                # tile-leading matmul needs the xt-DMA sem plus the
                # psum-slot-release sem. Emit a PE NoOp that declares those
                # data deps (1-elem APs, registered via annotate_deps) so
                # Tile's per-engine clock absorbs all waits there; the
                # matmuls then follow wait-free in PE program order. Tile
                # strips APs from InstNoOp at lowering, so walrus only
                # sees a plain NOP.
                gate = nc.tensor.nop(nofuse=True)
                gate.ins.ins = [nc.tensor.lower_ap(xt[0:1, 0, 0:1])]
                gate.ins.outs = [nc.tensor.lower_ap(ps[0:1, 0:1])]
                annotate_deps(tc.dep_state, gate.ins, tc.shadow_memory,
                              tc._rust_ctx, nc.inst_map)
                _reserve(nc, nc.tensor, gate, 4, prev=prev_mm)
                for c in range(KC):
                    mm = nc.tensor.matmul(
                        ps[:],
                        lhsT=xt[:, c, :],
                        rhs=wsb[:, c, :],
                        start=(c == 0),
                        stop=(c == KC - 1),
                    )
                prev_mm = mm

                # ---- scores / biased scores ----
                scores = wpool.tile([P, E], F32, tag="scores")
                sig = nc.scalar.activation(scores[:], ps[:], AF.Sigmoid)
                _reserve(nc, nc.scalar, sig, 3, prev=prev_sig)
                prev_sig = sig
                sfc = wpool.tile([P, E], F32, tag="sfc")
                badd = nc.vector.tensor_add(sfc[:], scores[:], bsb[:])
                _reserve(nc, nc.vector, badd, 3, prev=None)
                sfc3 = sfc[:].rearrange("p (g e) -> p g e", g=N_GROUP)

                # ---- group scores: sum of top-2 per group of 32 ----
                g3 = wpool.tile([P, N_GROUP, 8], F32, tag="g3")
                for g in range(N_GROUP):
                    nc.vector.max(g3[:, g, :], sfc[:, g * EG:(g + 1) * EG])
                gsum = wpool.tile([P, N_GROUP], F32, tag="gsum")
                nc.vector.tensor_add(gsum[:], g3[:, :, 0], g3[:, :, 1])

                # ---- pick top-4 groups; additive mask 0 / -BIG ----
                g8 = wpool.tile([P, 8], F32, tag="g8")
                nc.vector.max(g8[:], gsum[:])
                gneg = wpool.tile([P, N_GROUP], F32, tag="gneg")
                # (gsum < 4th-largest) * NEG -> 0 for kept groups, NEG else
                nc.vector.tensor_scalar(
                    gneg[:], gsum[:], g8[:, TOPK_GROUP - 1:TOPK_GROUP], NEG,
                    op0=OP.is_lt, op1=OP.mult,
                )

                # ---- masked biased scores; top-8 experts ----
                tmp = wpool.tile([P, E], F32, tag="tmp")
                tmp3 = tmp[:].rearrange("p (g e) -> p g e", g=N_GROUP)
                nc.vector.tensor_tensor(
                    tmp3, sfc3, gneg[:, :, None].to_broadcast([P, N_GROUP, EG]),
                    op=OP.add,
                )
                max8 = wpool.tile([P, 8], F32, tag="max8")
                nc.vector.max(max8[:], tmp[:])
                idx8 = idx_all[:, j, :]
                nc.vector.max_index(idx8, max8[:], tmp[:])

                # ---- selected-set mask via match_replace diff ----
                zap = wpool.tile([P, E], F32, tag="zap")
                nc.vector.match_replace(
                    zap[:], in_to_replace=max8[:], in_values=tmp[:], imm_value=NEG
                )
                sel = wpool.tile([P, E], U32, tag="sel")
                nc.vector.tensor_tensor(sel[:], tmp[:], zap[:], op=OP.not_equal)

                # ---- unbiased scores of the selected 8, sorted by score ----
                sm = wpool.tile([P, E], F32, tag="sm")
                nc.vector.memset(sm[:], NEG)
                nc.vector.copy_predicated(sm[:], sel[:], scores[:])
                smax8 = wpool.tile([P, 8], F32, tag="smax8")
                nc.vector.max(smax8[:], sm[:])
                sidx8 = wpool.tile([P, 8], U32, tag="sidx8")
                nc.vector.max_index(sidx8[:], smax8[:], sm[:])

                # ---- reorder scores to biased-rank order: w8[k] = sum_j
                #      smax8[j] * (sidx8[j] == idx8[k]) ----
                idxf = wpool.tile([P, 8], F32, tag="idxf")
                nc.vector.tensor_copy(idxf[:], idx8)
                sidxf = wpool.tile([P, 8], F32, tag="sidxf")
                nc.vector.tensor_copy(sidxf[:], sidx8[:])
                eq = wpool.tile([P, 8, 8], F32, tag="eq")
                nc.vector.tensor_tensor(
                    eq[:],
                    idxf[:, :, None].to_broadcast([P, 8, 8]),
                    sidxf[:, None, :].to_broadcast([P, 8, 8]),
                    op=OP.is_equal,
                )
                wprod = wpool.tile([P, 8, 8], F32, tag="wprod")
                nc.vector.tensor_tensor(
                    wprod[:], eq[:], smax8[:, None, :].to_broadcast([P, 8, 8]),
                    op=OP.mult,
                )
                w8 = wpool.tile([P, 8], F32, tag="w8")
                nc.vector.reduce_sum(w8[:], wprod[:], axis=mybir.AxisListType.X)

                # ---- normalize: w = 2.5 * w / (sum(w) + 1e-20) ----
                den = wpool.tile([P, 1], F32, tag="den")
                nc.vector.reduce_sum(den[:], w8[:], axis=mybir.AxisListType.X)
                nc.vector.tensor_scalar_add(den[:], den[:], 1e-20)
                rden = wpool.tile([P, 1], F32, tag="rden")
                nc.vector.reciprocal(rden[:], den[:])
                last_wout = nc.vector.tensor_scalar(
                    w_all[:, j, :], w8[:], rden[:], ROUTED_SCALING,
                    op0=OP.mult, op1=OP.mult,
                )

            d1 = nc.sync.dma_start(idx3, idx_all[:])
            _reserve(nc, nc.sync, d1, 2, prev=prev_dma)
            d2 = nc.sync.dma_start(w3, w_all[:])
            _reserve(nc, nc.sync, d2, 2, prev=d1)
            # Tail carriers: Tile's kernel-tail drain on SP waits on every
            # DMA queue sem (12 waits); give the legalizer enough nops.
            tail = d2.ins
            for _ in range(14):
                nop = nc.sync.nop(nofuse=True)
                add_dep_helper(nop.ins, tail, sync=False,
                               reason="tail drain wait carriers")
                tail = nop.ins

    _legalize_waits(nc)
    return nc


class _Runner:
    """Compile-once SPMD runner (mirrors bass2jax.run_bass_via_pjrt's
    multi-core path, but holds the jitted fn so repeated calls don't
    re-trace/re-jit; inputs can stay resident on device for timing)."""

    def __init__(self, nc):
        import jax
        from jax.experimental.shard_map import shard_map
        from jax.sharding import Mesh, NamedSharding, PartitionSpec

        from concourse import bass2jax

        bass2jax.install_neuronx_cc_hook()
        self._jax = jax
        self.nc = nc

        partition_name = (
            nc.partition_id_tensor.name if nc.partition_id_tensor else None
        )
        in_names, out_names, out_avals, zero_outs = [], [], [], []
        for alloc in nc.m.functions[0].allocations:
            if not isinstance(alloc, mybir.MemoryLocationSet):
                continue
            name = alloc.memorylocations[0].name
            if alloc.kind == "ExternalInput":
                if name != partition_name:
                    in_names.append(name)
            elif alloc.kind == "ExternalOutput":
                shape = tuple(alloc.tensor_shape)
                dtype = mybir.dt.np(alloc.dtype)
                out_names.append(name)
                out_avals.append(jax.core.ShapedArray(shape, dtype))
                zero_outs.append(np.zeros(shape, dtype))
        self.in_names = list(in_names)
        self.out_names = out_names
        self.out_avals = out_avals
        self.zero_outs = zero_outs
        n_params = len(in_names)
        self.n_params = n_params

        all_names = in_names + out_names
        if partition_name is not None:
            all_names.append(partition_name)

        def _body(*args):
            operands = list(args)
            if partition_name is not None:
                operands.append(bass2jax.partition_id_tensor())
            outs = bass2jax._bass_exec_p.bind(
                *operands,
                out_avals=tuple(out_avals),
                in_names=tuple(all_names),
                out_names=tuple(out_names),
                lowering_input_output_aliases=(),
                sim_require_finite=True,
                sim_require_nnan=True,
                nc=nc,
            )
            return tuple(outs)

        devices = jax.devices()[:N_CORES]
        assert len(devices) == N_CORES
        self.mesh = Mesh(np.asarray(devices), ("core",))
        n_outs = len(out_names)
        in_specs = (PartitionSpec("core"),) * (n_params + n_outs)
        out_specs = (PartitionSpec("core"),) * n_outs
        donate = tuple(range(n_params, n_params + n_outs))
        self._fn = jax.jit(
            shard_map(
                _body, mesh=self.mesh, in_specs=in_specs, out_specs=out_specs,
                check_rep=False,
            ),
            donate_argnums=donate,
            keep_unused=True,
        )
        self._sharding = NamedSharding(self.mesh, PartitionSpec("core"))

    def put_inputs(self, in_maps):
        """Concat per-core inputs on axis 0 and move to device once."""
        concat = [
            np.concatenate([np.asarray(m[name]) for m in in_maps], axis=0)
            for name in self.in_names
        ]
        return [self._jax.device_put(a, self._sharding) for a in concat]

    def _zeros(self):
        return [
            np.zeros((N_CORES * z.shape[0], *z.shape[1:]), z.dtype)
            for z in self.zero_outs
        ]

    def execute(self, dev_inputs):
        outs = self._fn(*dev_inputs, *self._zeros())
        self._jax.block_until_ready(outs)
        return outs

    def run(self, in_maps):
        dev_inputs = self.put_inputs(in_maps)
        out_arrs = self.execute(dev_inputs)
        return [
            {
                name: np.asarray(out_arrs[i]).reshape(
                    N_CORES, *self.out_avals[i].shape
                )[c]
                for i, name in enumerate(self.out_names)
            }
            for c in range(N_CORES)
        ]


_RUNNER_CACHE = {}


def _get_runner(mm_dt=MM_DT):
    if mm_dt not in _RUNNER_CACHE:
        _RUNNER_CACHE[mm_dt] = _Runner(build_nc(mm_dt))
    return _RUNNER_CACHE[mm_dt]


def make_in_maps(hidden_states, weight, e_score_correction_bias):
    x = np.ascontiguousarray(np.asarray(hidden_states), dtype=np.float32)
    x = x.reshape(T_FULL, H)
    w = np.asarray(weight, dtype=np.float32)
    b = np.asarray(e_score_correction_bias, dtype=np.float32)

    wT = np.ascontiguousarray(w.T)                       # [4096, 256]
    biasb = np.ascontiguousarray(np.broadcast_to(b, (P, E)))

    in_maps = []
    for i in range(N_CORES):
        xs = x[i * T_CORE:(i + 1) * T_CORE]
        in_maps.append({
            "xT": np.ascontiguousarray(xs.T),            # [4096, 2048]
            "wT": wT,
            "biasb": biasb,
        })
    return in_maps


def kernel(hidden_states, weight, e_score_correction_bias):
    runner = _get_runner()
    results = runner.run(
        make_in_maps(hidden_states, weight, e_score_correction_bias)
    )
    topk_idx = np.concatenate(
        [r["idx_out"].astype(np.int32) for r in results], axis=0
    )
    topk_weight = np.concatenate([r["w_out"] for r in results], axis=0)
    return topk_idx, topk_weight



# revision 2
# speedup vs baseline: 365.5747x; 365.5747x over previous
"""MiMo V2 MoE gate (sigmoid routing, grouped top-k) on 8 Trainium2 cores.

Contract: kernel(**inputs) takes the FULL unsharded inputs
(hidden_states [4,4096,4096] f32, weight [256,4096] f32,
e_score_correction_bias [256] f32) and returns (topk_idx int32 [16384,8],
topk_weight f32 [16384,8]) matching reference.py.

Strategy (data-parallel over tokens):
  - 16384 tokens are sharded 2048/core across 8 NeuronCores.
  - The gate GEMM runs as an exact-fp32 *split* product: x and w are
    decomposed on host into bf16 hi + bf16 lo parts (x = xh + xl,
    w = wh + wl exactly to ~2^-18 rel), and the device accumulates
    xh*wh + xh*wl + xl*wh in fp32 PSUM. Three 1-pass bf16 matmuls beat
    one 4-pass fp32 matmul by 25% on the PE, and the dropped xl*wl term
    is ~2^-18 relative -- measured 5/131072 idx flips (e_idx 1.4e-3).
  - Host pre-packs each x shard into per-tile PE layout
    [tile, 128 h-part, 32 h-chunk, 128 tok] so every DMA partition line
    is one contiguous 8KB run (the naive strided load runs 512B
    descriptors at ~150 GB/s; packed reaches full HBM rate).
  - Per core: 16 token tiles; per tile 96 matmuls accumulate the
    [128 tok, 256 expert] logits in PSUM, sigmoid on ScalarE, then the
    grouped top-k entirely on VectorE using the DVE sort8 primitives
    (max / max_index / match_replace). The topk weights (scores at the
    selected experts, ordered by biased-score rank) are recovered
    without any gather via an 8x8 index-match between the two sort
    orders.
"""

import sys

if "/opt/trn_rl_repo" not in sys.path:
    sys.path.insert(0, "/opt/trn_rl_repo")

import ml_dtypes
import numpy as np

import concourse.bass as bass
import concourse.mybir as mybir
import concourse.tile as tile
from concourse.tile_rust import add_dep_helper, annotate_deps

P = 128
H = 4096
E = 256
N_CORES = 8
T_FULL = 16384
T_CORE = T_FULL // N_CORES  # 2048
KC = H // P                 # 32 contraction chunks
TOK_TILES = T_CORE // P     # 16 token tiles per core
N_GROUP = 8
EG = E // N_GROUP           # 32 experts per group
TOPK_GROUP = 4
TOP_K = 8
ROUTED_SCALING = 2.5
NEG = -1e30

F32 = mybir.dt.float32
BF16 = mybir.dt.bfloat16
U32 = mybir.dt.uint32
AF = mybir.ActivationFunctionType
OP = mybir.AluOpType

BF16_NP = ml_dtypes.bfloat16


def _reserve(nc, eng, X, n, prev=None):
    """Emit n plain nops on X's engine, ordered after `prev` (a
    BassInstruction or None) and before X. They act as spare 1-wait
    carriers for _legalize_waits (every TPB instruction has exactly one
    HW wait slot; Tile can assign several waits to one instruction,
    which walrus then rejects)."""
    last = prev.ins if prev is not None else None
    for _ in range(n):
        nop = eng.nop(nofuse=True)
        if last is not None:
            add_dep_helper(nop.ins, last, sync=False,
                           reason="chain reserve nop after predecessor")
        add_dep_helper(X.ins, nop.ins, sync=False,
                       reason="reserve nop precedes its instruction")
        last = nop.ins


def _legalize_waits(nc, report=None):
    """Every TPB instruction has ONE hardware wait slot; Tile can assign
    several on_wait entries to an instruction, which walrus rejects
    ("Too many sync wait commands"). Fix in two ways, per engine stream
    (scheduled order):
      1. value-floor dedup: drop waits already implied by an earlier wait
         on the same semaphore in this stream (monotonic sems).
      2. excess-wait hoisting: move extra waits onto the nearest earlier
         wait-free instruction, scanning only across instructions with no
         on_update (pure nops) -- crossing an updater could reorder a
         producer chain and deadlock; this rule keeps placements provably
         safe. _reserve() plants such nops next to risky instructions.
    Drains are skipped (they encode multi-sem waits natively)."""
    stop_types = (
        mybir.InstDrain,
        mybir.InstEventSemaphore,
        mybir.InstCall,
    )
    leftover = []
    if True:
        # The kernel CFG is linear (main block -> end block), so per-engine
        # program order is the block-order concatenation. Crossing an
        # unconditional branch just means waiting before the jump.
        streams = {}
        nonmono = set()  # sems that ever decrease (barrier sems): no
                         # floor-dedup and no relocation for their waits
        for blk in nc.m.functions[0].blocks:
            for inst in blk.instructions:
                streams.setdefault(str(inst.engine), []).append(inst)
                si = inst.sync_info
                for u in (si.on_update if si and si.on_update else []):
                    if str(u.update_mode) not in ('sem-inc', 'sem-add-imm'):
                        nonmono.add(u.id)
        for stream in streams.values():
            floor = {}
            for i, X in enumerate(stream):
                si = X.sync_info
                if si is None or not si.on_wait:
                    continue
                mode_ok = lambda w: (str(w.wait_mode) == 'sem-ge-imm'
                                     and w.id not in nonmono)
                waits = []
                for w in si.on_wait:
                    if (mode_ok(w) and w.id in floor
                            and floor[w.id] >= w.wait_value):
                        continue  # already implied earlier in this stream
                    waits.append(w)
                moved = []
                if len(waits) > 1:
                    # only sem-ge waits are relocatable; sem-sub barrier
                    # ops must stay exactly where Tile put them
                    fixed = [w for w in waits if not mode_ok(w)]
                    movable = [w for w in waits if mode_ok(w)]
                    keep = fixed + movable[:max(0, 1 - len(fixed))]
                    maybe_move = movable[max(0, 1 - len(fixed)):]
                    for w in maybe_move:
                        placed = False
                        for k in range(i - 1, -1, -1):
                            C = stream[k]
                            if isinstance(C, stop_types):
                                break
                            csi = C.sync_info
                            if csi and csi.on_update:
                                break  # never cross a semaphore producer
                            cw = list(csi.on_wait) if csi and csi.on_wait else []
                            if cw or isinstance(
                                    C, mybir.InstUnconditionalBranch):
                                continue  # occupied/branch; keep scanning
                                          # (same-sequencer waits commute)
                            C.sync_info = mybir.SyncInfo(on_wait=[w],
                                                         on_update=[])
                            placed = True
                            break
                        if placed:
                            moved.append(w)
                        else:
                            keep.append(w)
                    waits = keep
                for w in list(waits) + moved:
                    if mode_ok(w):
                        floor[w.id] = max(floor.get(w.id, 0), w.wait_value)
                X.sync_info = mybir.SyncInfo(
                    on_wait=waits,
                    on_update=list(si.on_update) if si.on_update else [])
                if len(waits) > 1:
                    leftover.append((X.name, str(X.engine),
                                     type(X).__name__, len(waits)))
    # The PE gate ENGINE_NOPs carry AP operands purely for Tile dep
    # tracking; walrus's engine check rejects a nop with operands, so
    # strip them now (tile.py does the same for InstNoOp instructions).
    for blk in nc.m.functions[0].blocks:
        for inst in blk.instructions:
            if (isinstance(inst, mybir.InstISA) and (inst.ins or inst.outs)
                    and inst.op_name == 'ENGINE_NOP'):
                inst.ins = []
                inst.outs = []

    if report is not None:
        report.extend(leftover)
    elif leftover:
        raise RuntimeError(f"wait legalization failed for: {leftover}")


def build_nc():
    nc = bass.Bass()

    # Host-packed per-tile x layouts: [(j p), (c t)] so that tile j is a
    # [128, 4096] slab with an 8KB contiguous line per partition.
    xhi = nc.dram_tensor("xhi", [TOK_TILES * P, KC * P], BF16, kind="ExternalInput")
    xlo = nc.dram_tensor("xlo", [TOK_TILES * P, KC * P], BF16, kind="ExternalInput")
    # Host-packed weights: [p, (c e)] -> [128, 8192], 16KB/partition.
    whi = nc.dram_tensor("whi", [P, KC * E], BF16, kind="ExternalInput")
    wlo = nc.dram_tensor("wlo", [P, KC * E], BF16, kind="ExternalInput")
    biasb = nc.dram_tensor("biasb", [P, E], F32, kind="ExternalInput")
    idx_out = nc.dram_tensor("idx_out", [T_CORE, TOP_K], U32, kind="ExternalOutput")
    w_out = nc.dram_tensor("w_out", [T_CORE, TOP_K], F32, kind="ExternalOutput")

    xhi4 = xhi.ap().rearrange("(j p) (c t) -> p j c t", p=P, c=KC)
    xlo4 = xlo.ap().rearrange("(j p) (c t) -> p j c t", p=P, c=KC)
    whi3 = whi.ap().rearrange("p (c e) -> p c e", c=KC)
    wlo3 = wlo.ap().rearrange("p (c e) -> p c e", c=KC)
    idx3 = idx_out.ap().rearrange("(j p) k -> p j k", p=P)  # [128, 16, 8]
    w3 = w_out.ap().rearrange("(j p) k -> p j k", p=P)

    with tile.TileContext(nc) as tc:
        with (
            tc.tile_pool(name="const", bufs=1) as cpool,
            # bufs=8 so an xt slot's previous DMA sits a full queue
            # round-robin cycle back -> same HWDGE queue -> WAW covered by
            # queue FIFO, leaving each xt DMA a single (PE slot-release)
            # wait.
            tc.tile_pool(name="xhin", bufs=8) as xhpool,
            tc.tile_pool(name="xlin", bufs=8) as xlpool,
            tc.tile_pool(name="psum", bufs=2, space="PSUM") as pspool,
            tc.tile_pool(name="work", bufs=2) as wpool,
        ):
            wh = cpool.tile([P, KC, E], BF16)
            nc.sync.dma_start(wh[:], whi3)
            wl = cpool.tile([P, KC, E], BF16)
            nc.sync.dma_start(wl[:], wlo3)
            bsb = cpool.tile([P, E], F32)
            nc.sync.dma_start(bsb[:], biasb.ap())
            # Persistent per-core output accumulators: no slot reuse, so
            # the DVE producers of idx/w never wait on output DMAs.
            idx_all = cpool.tile([P, TOK_TILES, TOP_K], U32)
            w_all = cpool.tile([P, TOK_TILES, TOP_K], F32)

            prev_sig = None
            prev_mm = None
            prev_dma = None
            last_wout = None
            for j in range(TOK_TILES):
                # ---- gate GEMM: logits[128 tok, 256 exp] ----
                xh = xhpool.tile([P, KC, P], BF16, tag="xh")
                xh_dma = nc.sync.dma_start(xh[:], xhi4[:, j])
                _reserve(nc, nc.sync, xh_dma, 3, prev=prev_dma)
                xl = xlpool.tile([P, KC, P], BF16, tag="xl")
                xl_dma = nc.sync.dma_start(xl[:], xlo4[:, j])
                _reserve(nc, nc.sync, xl_dma, 3, prev=xh_dma)
                prev_dma = xl_dma
                ps = pspool.tile([P, E], F32, tag="ps")
                # The fused matmul (self-loading LDWEIGHTS) only has budget
                # for ONE semaphore wait in walrus codegen, but the
                # tile-leading matmul needs the xh/xl-DMA sems plus the
                # psum-slot-release sem. Emit a PE NoOp that declares those
                # data deps (1-elem APs, registered via annotate_deps) so
                # Tile's per-engine clock absorbs all waits there; the
                # matmuls then follow wait-free in PE program order. Tile
                # strips APs from InstNoOp at lowering, so walrus only
                # sees a plain NOP.
                gate = nc.tensor.nop(nofuse=True)
                gate.ins.ins = [
                    nc.tensor.lower_ap(xh[0:1, 0, 0:1]),
                    nc.tensor.lower_ap(xl[0:1, 0, 0:1]),
                ]
                gate.ins.outs = [nc.tensor.lower_ap(ps[0:1, 0:1])]
                annotate_deps(tc.dep_state, gate.ins, tc.shadow_memory,
                              tc._rust_ctx, nc.inst_map)
                _reserve(nc, nc.tensor, gate, 4, prev=prev_mm)
                # Exact-fp32 split product: ps = xh*wh + xh*wl + xl*wh,
                # all accumulated in fp32 PSUM.
                for c in range(KC):
                    nc.tensor.matmul(
                        ps[:], lhsT=xh[:, c, :], rhs=wh[:, c, :],
                        start=(c == 0), stop=False,
                    )
                    nc.tensor.matmul(
                        ps[:], lhsT=xh[:, c, :], rhs=wl[:, c, :],
                        start=False, stop=False,
                    )
                    mm = nc.tensor.matmul(
                        ps[:], lhsT=xl[:, c, :], rhs=wh[:, c, :],
                        start=False, stop=(c == KC - 1),
                    )
                prev_mm = mm

                # ---- scores / biased scores ----
                scores = wpool.tile([P, E], F32, tag="scores")
                sig = nc.scalar.activation(scores[:], ps[:], AF.Sigmoid)
                _reserve(nc, nc.scalar, sig, 3, prev=prev_sig)
                prev_sig = sig
                sfc = wpool.tile([P, E], F32, tag="sfc")
                badd = nc.vector.tensor_add(sfc[:], scores[:], bsb[:])
                _reserve(nc, nc.vector, badd, 3, prev=None)
                sfc3 = sfc[:].rearrange("p (g e) -> p g e", g=N_GROUP)

                # ---- group scores: sum of top-2 per group of 32 ----
                g3 = wpool.tile([P, N_GROUP, 8], F32, tag="g3")
                for g in range(N_GROUP):
                    nc.vector.max(g3[:, g, :], sfc[:, g * EG:(g + 1) * EG])
                gsum = wpool.tile([P, N_GROUP], F32, tag="gsum")
                nc.vector.tensor_add(gsum[:], g3[:, :, 0], g3[:, :, 1])

                # ---- pick top-4 groups; additive mask 0 / -BIG ----
                g8 = wpool.tile([P, 8], F32, tag="g8")
                nc.vector.max(g8[:], gsum[:])
                gneg = wpool.tile([P, N_GROUP], F32, tag="gneg")
                # (gsum < 4th-largest) * NEG -> 0 for kept groups, NEG else
                nc.vector.tensor_scalar(
                    gneg[:], gsum[:], g8[:, TOPK_GROUP - 1:TOPK_GROUP], NEG,
                    op0=OP.is_lt, op1=OP.mult,
                )

                # ---- masked biased scores; top-8 experts ----
                tmp = wpool.tile([P, E], F32, tag="tmp")
                tmp3 = tmp[:].rearrange("p (g e) -> p g e", g=N_GROUP)
                nc.vector.tensor_tensor(
                    tmp3, sfc3, gneg[:, :, None].to_broadcast([P, N_GROUP, EG]),
                    op=OP.add,
                )
                max8 = wpool.tile([P, 8], F32, tag="max8")
                nc.vector.max(max8[:], tmp[:])
                idx8 = idx_all[:, j, :]
                nc.vector.max_index(idx8, max8[:], tmp[:])

                # ---- selected-set mask via match_replace diff ----
                zap = wpool.tile([P, E], F32, tag="zap")
                nc.vector.match_replace(
                    zap[:], in_to_replace=max8[:], in_values=tmp[:], imm_value=NEG
                )
                sel = wpool.tile([P, E], U32, tag="sel")
                nc.vector.tensor_tensor(sel[:], tmp[:], zap[:], op=OP.not_equal)

                # ---- unbiased scores of the selected 8, sorted by score ----
                sm = wpool.tile([P, E], F32, tag="sm")
                nc.vector.memset(sm[:], NEG)
                nc.vector.copy_predicated(sm[:], sel[:], scores[:])
                smax8 = wpool.tile([P, 8], F32, tag="smax8")
                nc.vector.max(smax8[:], sm[:])
                sidx8 = wpool.tile([P, 8], U32, tag="sidx8")
                nc.vector.max_index(sidx8[:], smax8[:], sm[:])

                # ---- reorder scores to biased-rank order: w8[k] = sum_j
                #      smax8[j] * (sidx8[j] == idx8[k]) ----
                idxf = wpool.tile([P, 8], F32, tag="idxf")
                nc.vector.tensor_copy(idxf[:], idx8)
                sidxf = wpool.tile([P, 8], F32, tag="sidxf")
                nc.vector.tensor_copy(sidxf[:], sidx8[:])
                eq = wpool.tile([P, 8, 8], F32, tag="eq")
                nc.vector.tensor_tensor(
                    eq[:],
                    idxf[:, :, None].to_broadcast([P, 8, 8]),
                    sidxf[:, None, :].to_broadcast([P, 8, 8]),
                    op=OP.is_equal,
                )
                wprod = wpool.tile([P, 8, 8], F32, tag="wprod")
                nc.vector.tensor_tensor(
                    wprod[:], eq[:], smax8[:, None, :].to_broadcast([P, 8, 8]),
                    op=OP.mult,
                )
                w8 = wpool.tile([P, 8], F32, tag="w8")
                nc.vector.reduce_sum(w8[:], wprod[:], axis=mybir.AxisListType.X)

                # ---- normalize: w = 2.5 * w / (sum(w) + 1e-20) ----
                den = wpool.tile([P, 1], F32, tag="den")
                nc.vector.reduce_sum(den[:], w8[:], axis=mybir.AxisListType.X)
                nc.vector.tensor_scalar_add(den[:], den[:], 1e-20)
                rden = wpool.tile([P, 1], F32, tag="rden")
                nc.vector.reciprocal(rden[:], den[:])
                last_wout = nc.vector.tensor_scalar(
                    w_all[:, j, :], w8[:], rden[:], ROUTED_SCALING,
                    op0=OP.mult, op1=OP.mult,
                )

            d1 = nc.sync.dma_start(idx3, idx_all[:])
            _reserve(nc, nc.sync, d1, 2, prev=prev_dma)
            d2 = nc.sync.dma_start(w3, w_all[:])
            _reserve(nc, nc.sync, d2, 2, prev=d1)
            # Tail carriers: Tile's kernel-tail drain on SP waits on every
            # DMA queue sem (12 waits); give the legalizer enough nops.
            tail = d2.ins
            for _ in range(14):
                nop = nc.sync.nop(nofuse=True)
                add_dep_helper(nop.ins, tail, sync=False,
                               reason="tail drain wait carriers")
                tail = nop.ins

    _legalize_waits(nc)
    return nc


class _Runner:
    """Compile-once SPMD runner (mirrors bass2jax.run_bass_via_pjrt's
    multi-core path, but holds the jitted fn so repeated calls don't
    re-trace/re-jit; inputs can stay resident on device for timing)."""

    def __init__(self, nc):
        import jax
        from jax.experimental.shard_map import shard_map
        from jax.sharding import Mesh, NamedSharding, PartitionSpec

        from concourse import bass2jax

        bass2jax.install_neuronx_cc_hook()
        self._jax = jax
        self.nc = nc

        partition_name = (
            nc.partition_id_tensor.name if nc.partition_id_tensor else None
        )
        in_names, out_names, out_avals, zero_outs = [], [], [], []
        for alloc in nc.m.functions[0].allocations:
            if not isinstance(alloc, mybir.MemoryLocationSet):
                continue
            name = alloc.memorylocations[0].name
            if alloc.kind == "ExternalInput":
                if name != partition_name:
                    in_names.append(name)
            elif alloc.kind == "ExternalOutput":
                shape = tuple(alloc.tensor_shape)
                dtype = mybir.dt.np(alloc.dtype)
                out_names.append(name)
                out_avals.append(jax.core.ShapedArray(shape, dtype))
                zero_outs.append(np.zeros(shape, dtype))
        self.in_names = list(in_names)
        self.out_names = out_names
        self.out_avals = out_avals
        self.zero_outs = zero_outs
        n_params = len(in_names)
        self.n_params = n_params

        all_names = in_names + out_names
        if partition_name is not None:
            all_names.append(partition_name)

        def _body(*args):
            operands = list(args)
            if partition_name is not None:
                operands.append(bass2jax.partition_id_tensor())
            outs = bass2jax._bass_exec_p.bind(
                *operands,
                out_avals=tuple(out_avals),
                in_names=tuple(all_names),
                out_names=tuple(out_names),
                lowering_input_output_aliases=(),
                sim_require_finite=True,
                sim_require_nnan=True,
                nc=nc,
            )
            return tuple(outs)

        devices = jax.devices()[:N_CORES]
        assert len(devices) == N_CORES
        self.mesh = Mesh(np.asarray(devices), ("core",))
        n_outs = len(out_names)
        in_specs = (PartitionSpec("core"),) * (n_params + n_outs)
        out_specs = (PartitionSpec("core"),) * n_outs
        donate = tuple(range(n_params, n_params + n_outs))
        self._fn = jax.jit(
            shard_map(
                _body, mesh=self.mesh, in_specs=in_specs, out_specs=out_specs,
                check_rep=False,
            ),
            donate_argnums=donate,
            keep_unused=True,
        )
        self._sharding = NamedSharding(self.mesh, PartitionSpec("core"))

    def put_inputs(self, in_maps):
        """Concat per-core inputs on axis 0 and move to device once."""
        concat = [
            np.concatenate([np.asarray(m[name]) for m in in_maps], axis=0)
            for name in self.in_names
        ]
        return [self._jax.device_put(a, self._sharding) for a in concat]

    def _zeros(self):
        return [
            np.zeros((N_CORES * z.shape[0], *z.shape[1:]), z.dtype)
            for z in self.zero_outs
        ]

    def execute(self, dev_inputs):
        outs = self._fn(*dev_inputs, *self._zeros())
        self._jax.block_until_ready(outs)
        return outs

    def run(self, in_maps):
        dev_inputs = self.put_inputs(in_maps)
        out_arrs = self.execute(dev_inputs)
        return [
            {
                name: np.asarray(out_arrs[i]).reshape(
                    N_CORES, *self.out_avals[i].shape
                )[c]
                for i, name in enumerate(self.out_names)
            }
            for c in range(N_CORES)
        ]


_RUNNER_CACHE = {}


def _get_runner():
    if "nc" not in _RUNNER_CACHE:
        _RUNNER_CACHE["nc"] = _Runner(build_nc())
    return _RUNNER_CACHE["nc"]


def make_in_maps(hidden_states, weight, e_score_correction_bias):
    x = np.ascontiguousarray(np.asarray(hidden_states), dtype=np.float32)
    x = x.reshape(T_FULL, H)
    w = np.asarray(weight, dtype=np.float32)
    b = np.asarray(e_score_correction_bias, dtype=np.float32)

    # Exact split: v = hi + lo with hi = bf16(v), lo = bf16(v - hi).
    x_hi = x.astype(BF16_NP)
    x_lo = (x - x_hi.astype(np.float32)).astype(BF16_NP)
    w_hi = w.astype(BF16_NP)
    w_lo = (w - w_hi.astype(np.float32)).astype(BF16_NP)

    def pack_x(xs):
        # [2048, 4096] -> [j, t, c, p] -> [j, p, c, t] -> [(j p), (c t)]
        return np.ascontiguousarray(
            xs.reshape(TOK_TILES, P, KC, P).transpose(0, 3, 2, 1)
        ).reshape(TOK_TILES * P, KC * P)

    def pack_w(ws):
        # [256, 4096] -> [h, e] -> [c, p, e] -> [p, (c e)]
        return np.ascontiguousarray(
            ws.T.reshape(KC, P, E).transpose(1, 0, 2)
        ).reshape(P, KC * E)

    whi = pack_w(w_hi)
    wlo = pack_w(w_lo)
    biasb = np.ascontiguousarray(np.broadcast_to(b, (P, E)))

    in_maps = []
    for i in range(N_CORES):
        sl = slice(i * T_CORE, (i + 1) * T_CORE)
        in_maps.append({
            "xhi": pack_x(x_hi[sl]),
            "xlo": pack_x(x_lo[sl]),
            "whi": whi,
            "wlo": wlo,
            "biasb": biasb,
        })
    return in_maps


def kernel(hidden_states, weight, e_score_correction_bias):
    runner = _get_runner()
    results = runner.run(
        make_in_maps(hidden_states, weight, e_score_correction_bias)
    )
    topk_idx = np.concatenate(
        [r["idx_out"].astype(np.int32) for r in results], axis=0
    )
    topk_weight = np.concatenate([r["w_out"] for r in results], axis=0)
    return topk_idx, topk_weight


# revision 19
# speedup vs baseline: 472.9055x; 1.2936x over previous
"""MiMo V2 MoE gate (sigmoid routing, grouped top-k) on 8 Trainium2 cores.

Contract: kernel(**inputs) takes the FULL unsharded inputs
(hidden_states [4,4096,4096] f32, weight [256,4096] f32,
e_score_correction_bias [256] f32) and returns (topk_idx int32 [16384,8],
topk_weight f32 [16384,8]) matching reference.py.

Strategy (data-parallel over tokens, 2048/core):
  - The gate GEMM runs as an exact-enough split product (measured
    7/131072 idx flips, e_idx 4.8e-3):
        logits = xh@wh  +  2^-16 * (s1*xl @ s2*wh  +  s3*xh @ s4*wl)
    with xh=fp16(x), xl=x-xh, wh=fp16(w), wl=w-wh. The main product is
    one fp16 matmul pass; the two correction products run as fp8e4m3
    DoubleRow matmuls (2 contraction chunks per instruction, 2x ALU),
    with power-of-2 scales s1*s2 == s3*s4 == 2^16 chosen to center each
    operand in fp8 range. Main and correction interleave per chunk so
    the PE's LDWEIGHTS reorder window hides the DoubleRow weight-load
    tax behind fp16 matmul streaming.
  - Host pre-packs every x operand into per-tile PE layout
    [tile, 128 h-part, chunk, 128 tok] so each DMA partition line is one
    contiguous run (strided loads ran 512B descriptors at ~40% of HBM
    rate).
  - Per tile: PSUM accumulates main [128,256] and correction [128,256];
    GpSimd fuses them (psc*2^-16 + ps), ScalarE applies sigmoid, and the
    grouped top-k runs on VectorE via DVE sort8 primitives
    (max / max_index / match_replace), with the bias-add and
    selected-score masking offloaded to GpSimd. The topk weights
    (scores at the selected experts, ordered by biased-score rank) are
    recovered without a gather via an 8x8 index-match between the two
    sort orders.
"""

import sys

if "/opt/trn_rl_repo" not in sys.path:
    sys.path.insert(0, "/opt/trn_rl_repo")

import ml_dtypes
import numpy as np

import concourse.bass as bass
import concourse.mybir as mybir
import concourse.tile as tile
from concourse.tile_rust import add_dep_helper, annotate_deps

P = 128
H = 4096
E = 256
N_CORES = 8
T_FULL = 16384
T_CORE = T_FULL // N_CORES  # 2048
KC = H // P                 # 32 contraction chunks
TOK_TILES = T_CORE // P     # 16 token tiles per core
N_GROUP = 8
EG = E // N_GROUP           # 32 experts per group
TOPK_GROUP = 4
TOP_K = 8
ROUTED_SCALING = 2.5
NEG = -1e30

F32 = mybir.dt.float32
F16 = mybir.dt.float16
F8 = mybir.dt.float8e4
U32 = mybir.dt.uint32
AF = mybir.ActivationFunctionType
OP = mybir.AluOpType

F16_NP = np.float16
F8_NP = ml_dtypes.float8_e4m3

# Correction scales: corr_psum = S * (xl@wh + xh@wl), S = 2^16.
S1 = 2.0 ** 11   # xl pre-scale
S2 = 2.0 ** 5    # wh pre-scale (pairs with xl)
S3 = 2.0 ** 0    # xh pre-scale
S4 = 2.0 ** 16   # wl pre-scale (pairs with xh)
CORR_SCALE = 2.0 ** -16


def _reserve(nc, eng, X, n, prev=None):
    """Emit n plain nops on X's engine, ordered after `prev` (a
    BassInstruction or None) and before X. They act as spare 1-wait
    carriers for _legalize_waits (every TPB instruction has exactly one
    HW wait slot; Tile can assign several waits to one instruction,
    which walrus then rejects)."""
    last = prev.ins if prev is not None else None
    for _ in range(n):
        nop = eng.nop(nofuse=True)
        if last is not None:
            add_dep_helper(nop.ins, last, sync=False,
                           reason="chain reserve nop after predecessor")
        add_dep_helper(X.ins, nop.ins, sync=False,
                       reason="reserve nop precedes its instruction")
        last = nop.ins


def _legalize_waits(nc, report=None):
    """Every TPB instruction has ONE hardware wait slot; Tile can assign
    several on_wait entries to an instruction, which walrus rejects
    ("Too many sync wait commands"). Fix in two ways, per engine stream
    (scheduled order):
      1. value-floor dedup: drop waits already implied by an earlier wait
         on the same semaphore in this stream (monotonic sems).
      2. excess-wait hoisting: move extra waits onto the nearest earlier
         wait-free instruction, scanning only across instructions with no
         on_update (pure nops) -- crossing an updater could reorder a
         producer chain and deadlock; this rule keeps placements provably
         safe. _reserve() plants such nops next to risky instructions.
    Drains are skipped (they encode multi-sem waits natively)."""
    leftover = []
    if True:
        # The kernel CFG is linear (main block -> end block), so per-engine
        # program order is the block-order concatenation.
        nonmono = set()  # sems that ever decrease (barrier sems): no
                         # floor-dedup for their waits
        for blk in nc.m.functions[0].blocks:
            for inst in blk.instructions:
                si = inst.sync_info
                for u in (si.on_update if si and si.on_update else []):
                    if str(u.update_mode) not in ('sem-inc', 'sem-add-imm'):
                        nonmono.add(u.id)
        # Pass 1: per-engine value-floor dedup of monotonic sem-ge waits.
        floors = {}
        for blk in nc.m.functions[0].blocks:
            for X in blk.instructions:
                si = X.sync_info
                if si is None or not si.on_wait:
                    continue
                floor = floors.setdefault(str(X.engine), {})
                mode_ok = lambda w: (str(w.wait_mode) == 'sem-ge-imm'
                                     and w.id not in nonmono)
                waits = []
                for w in si.on_wait:
                    if (mode_ok(w) and w.id in floor
                            and floor[w.id] >= w.wait_value):
                        continue  # already implied earlier in this stream
                    waits.append(w)
                    if mode_ok(w):
                        floor[w.id] = max(floor.get(w.id, 0), w.wait_value)
                X.sync_info = mybir.SyncInfo(
                    on_wait=waits,
                    on_update=list(si.on_update) if si.on_update else [])
        # Pass 2: any instruction still holding >1 waits gets all but one
        # moved onto fresh carrier nops inserted DIRECTLY before it in the
        # block (post-scheduling, so adjacency is guaranteed and the waits
        # execute at the same engine-stream position — semantically
        # identical to multiple waits on one instruction). This includes
        # the kernel-tail drains: their native multi-sem budget overflows
        # with this many DMA queues.
        carrier_id = [0]
        for blk in nc.m.functions[0].blocks:
            inserts = []
            for i, X in enumerate(blk.instructions):
                si = X.sync_info
                if si is None or not si.on_wait or len(si.on_wait) <= 1:
                    continue
                waits = list(si.on_wait)
                extra, keep = waits[:-1], waits[-1:]
                nops = []
                for w in extra:
                    nop = mybir.InstNoOp(
                        name=f"LW-{carrier_id[0]}", ins=[], outs=[])
                    carrier_id[0] += 1
                    nop.engine = X.engine
                    nop.bass_nofuse = True
                    nop.sync_info = mybir.SyncInfo(on_wait=[w], on_update=[])
                    nops.append(nop)
                inserts.append((i, nops))
                X.sync_info = mybir.SyncInfo(
                    on_wait=keep,
                    on_update=list(si.on_update) if si.on_update else [])
            for i, nops in reversed(inserts):
                blk.instructions[i:i] = nops
    # The PE gate ENGINE_NOPs carry AP operands purely for Tile dep
    # tracking; walrus's engine check rejects a nop with operands, so
    # strip them now (tile.py does the same for InstNoOp instructions).
    for blk in nc.m.functions[0].blocks:
        for inst in blk.instructions:
            if (isinstance(inst, mybir.InstISA) and (inst.ins or inst.outs)
                    and inst.op_name == 'ENGINE_NOP'):
                inst.ins = []
                inst.outs = []

    if report is not None:
        report.extend(leftover)
    elif leftover:
        raise RuntimeError(f"wait legalization failed for: {leftover}")


def build_nc():
    nc = bass.Bass()

    # Host-packed per-tile x layouts: [(j p), (c t)] so tile j is a
    # [128, KC*128] slab with one contiguous line per partition.
    xh16 = nc.dram_tensor("xh16", [TOK_TILES * P, KC * P], F16, kind="ExternalInput")
    xl8 = nc.dram_tensor("xl8", [TOK_TILES * P, KC * P], F8, kind="ExternalInput")
    xh8 = nc.dram_tensor("xh8", [TOK_TILES * P, KC * P], F8, kind="ExternalInput")
    # Host-packed weights: [p, (c e)].
    wh16 = nc.dram_tensor("wh16", [P, KC * E], F16, kind="ExternalInput")
    # wc8 carries 2*KC chunks: chunks 0..31 = s2*wh, 32..63 = s4*wl.
    wc8 = nc.dram_tensor("wc8", [P, 2 * KC * E], F8, kind="ExternalInput")
    biasb = nc.dram_tensor("biasb", [P, E], F32, kind="ExternalInput")
    idx_out = nc.dram_tensor("idx_out", [T_CORE, TOP_K], U32, kind="ExternalOutput")
    w_out = nc.dram_tensor("w_out", [T_CORE, TOP_K], F32, kind="ExternalOutput")

    xh16_4 = xh16.ap().rearrange("(j p) (c t) -> p j c t", p=P, c=KC)
    xl8_4 = xl8.ap().rearrange("(j p) (c t) -> p j c t", p=P, c=KC)
    xh8_4 = xh8.ap().rearrange("(j p) (c t) -> p j c t", p=P, c=KC)
    wh16_3 = wh16.ap().rearrange("p (c e) -> p c e", c=KC)
    wc8_3 = wc8.ap().rearrange("p (c e) -> p c e", c=2 * KC)
    idx3 = idx_out.ap().rearrange("(j p) k -> p j k", p=P)  # [128, 16, 8]
    w3 = w_out.ap().rearrange("(j p) k -> p j k", p=P)

    with tile.TileContext(nc) as tc:
        with (
            tc.tile_pool(name="const", bufs=1) as cpool,
            tc.tile_pool(name="xhin", bufs=8) as xhpool,
            tc.tile_pool(name="xl8in", bufs=8) as xl8pool,
            tc.tile_pool(name="xh8in", bufs=8) as xh8pool,
            tc.tile_pool(name="psum", bufs=2, space="PSUM") as pspool,
            tc.tile_pool(name="work", bufs=2) as wpool,
        ):
            # DMA order: wh16 then tile-0 x parts before the big wc8 so
            # tile 0's matmuls can start as early as possible.
            wh = cpool.tile([P, KC, E], F16)
            nc.sync.dma_start(wh[:], wh16_3)
            wc = cpool.tile([P, 2 * KC, E], F8)
            bsb = cpool.tile([P, E], F32)
            idx_all = cpool.tile([P, TOK_TILES, TOP_K], U32)
            w_all = cpool.tile([P, TOK_TILES, TOP_K], F32)

            prev_sig = None
            prev_mm = None
            prev_dma = None
            prev_gp = None
            prev_dve = None
            for j in range(TOK_TILES):
                xh = xhpool.tile([P, KC, P], F16, tag="xh")
                xh_dma = nc.sync.dma_start(xh[:], xh16_4[:, j])
                _reserve(nc, nc.sync, xh_dma, 3, prev=prev_dma)
                xl = xl8pool.tile([P, KC, P], F8, tag="xl")
                xl_dma = nc.sync.dma_start(xl[:], xl8_4[:, j])
                _reserve(nc, nc.sync, xl_dma, 3, prev=xh_dma)
                x8 = xh8pool.tile([P, KC, P], F8, tag="x8")
                x8_dma = nc.sync.dma_start(x8[:], xh8_4[:, j])
                _reserve(nc, nc.sync, x8_dma, 3, prev=xl_dma)
                prev_dma = x8_dma
                if j == 0:
                    wc_dma = nc.sync.dma_start(wc[:], wc8_3)
                    _reserve(nc, nc.sync, wc_dma, 2, prev=prev_dma)
                    b_dma = nc.sync.dma_start(bsb[:], biasb.ap())
                    _reserve(nc, nc.sync, b_dma, 2, prev=wc_dma)
                    prev_dma = b_dma

                ps = pspool.tile([P, E], F32, tag="ps")
                psc = pspool.tile([P, E], F32, tag="psc")
                # The fused matmul (self-loading LDWEIGHTS) only has budget
                # for ONE semaphore wait in walrus codegen, but the
                # tile-leading matmul needs the x-DMA sems plus the
                # psum-slot-release sems. Emit a PE NoOp that declares those
                # data deps (1-elem APs, registered via annotate_deps) so
                # Tile's per-engine clock absorbs all waits there; the
                # matmuls then follow wait-free in PE program order. Tile
                # strips APs from InstNoOp at lowering, so walrus only
                # sees a plain NOP.
                gate = nc.tensor.nop(nofuse=True)
                gate.ins.ins = [nc.tensor.lower_ap(xh[0:1, 0, 0:1])]
                gate.ins.outs = [nc.tensor.lower_ap(ps[0:1, 0:1])]
                annotate_deps(tc.dep_state, gate.ins, tc.shadow_memory,
                              tc._rust_ctx, nc.inst_map)
                _reserve(nc, nc.tensor, gate, 4, prev=prev_mm)
                # Main fp16 pass (one contiguous accumulation group —
                # interleaving two groups miscompiles; the PE's 64-deep
                # reorder window still lets the correction group's
                # LDWEIGHTS creep into this stream).
                for i in range(KC):
                    nc.tensor.matmul(
                        ps[:], lhsT=xh[:, i, :], rhs=wh[:, i, :],
                        start=(i == 0), stop=(i == KC - 1),
                    )
                # Correction fp8 DoubleRow pass: pair i contracts chunks
                # (2i, 2i+1); pairs 0..15 are the xl-block, 16..31 the
                # xh-block.
                gate2 = nc.tensor.nop(nofuse=True)
                gate2.ins.ins = [
                    nc.tensor.lower_ap(xl[0:1, 0, 0:1]),
                    nc.tensor.lower_ap(x8[0:1, 0, 0:1]),
                ]
                gate2.ins.outs = [nc.tensor.lower_ap(psc[0:1, 0:1])]
                annotate_deps(tc.dep_state, gate2.ins, tc.shadow_memory,
                              tc._rust_ctx, nc.inst_map)
                _reserve(nc, nc.tensor, gate2, 4, prev=gate)
                for i in range(KC):
                    if i < KC // 2:
                        clhs, coff = xl, 2 * i
                    else:
                        clhs, coff = x8, 2 * (i - KC // 2)
                    mm = nc.tensor.matmul(
                        psc[:],
                        lhsT=clhs[:, coff:coff + 2, :],
                        rhs=wc[:, 2 * i:2 * i + 2, :],
                        start=(i == 0), stop=(i == KC - 1),
                        perf_mode=mybir.MatmulPerfMode.DoubleRow,
                    )
                prev_mm = mm

                # ---- logits = ps + 2^-16 * psc, sigmoid ----
                # Engines may read only ONE input from PSUM per instruction
                # (and GpSimd none), so: ACT scaled-copies psc to SBUF
                # (Copy bypasses the activation table, so no table thrash
                # with Sigmoid), then DVE adds ps (PSUM) + cb (SBUF).
                cb = wpool.tile([P, E], F32, tag="cb")
                cp = nc.scalar.activation(cb[:], psc[:], AF.Copy,
                                          scale=CORR_SCALE)
                _reserve(nc, nc.scalar, cp, 3, prev=prev_sig)
                u = wpool.tile([P, E], F32, tag="u")
                stt = nc.vector.tensor_add(u[:], ps[:], cb[:])
                _reserve(nc, nc.vector, stt, 3, prev=prev_dve)
                scores = wpool.tile([P, E], F32, tag="scores")
                sig = nc.scalar.activation(scores[:], u[:], AF.Sigmoid)
                _reserve(nc, nc.scalar, sig, 2, prev=cp)
                prev_sig = sig
                sfc = wpool.tile([P, E], F32, tag="sfc")
                badd = nc.gpsimd.tensor_add(sfc[:], scores[:], bsb[:])
                _reserve(nc, nc.gpsimd, badd, 2, prev=prev_gp)
                sfc3 = sfc[:].rearrange("p (g e) -> p g e", g=N_GROUP)

                # ---- group scores: sum of top-2 per group of 32 ----
                # max1 = per-group max; zap first occurrence of each (the 8
                # group maxes are bitwise-distinct for random data); max2 =
                # per-group max of the rest; gsum = max1 + max2.
                gmax1 = wpool.tile([P, N_GROUP], F32, tag="gmax1")
                nc.vector.reduce_max(gmax1[:], sfc3, axis=mybir.AxisListType.X)
                zapg = wpool.tile([P, E], F32, tag="zapg")
                nc.vector.match_replace(
                    zapg[:], in_to_replace=gmax1[:], in_values=sfc[:],
                    imm_value=NEG,
                )
                zapg3 = zapg[:].rearrange("p (g e) -> p g e", g=N_GROUP)
                gsum = wpool.tile([P, N_GROUP], F32, tag="gsum")
                nc.vector.reduce_max(gsum[:], zapg3, axis=mybir.AxisListType.X)
                nc.vector.tensor_add(gsum[:], gsum[:], gmax1[:])

                # ---- pick top-4 groups; additive mask 0 / -BIG ----
                g8 = wpool.tile([P, 8], F32, tag="g8")
                nc.vector.max(g8[:], gsum[:])
                gneg = wpool.tile([P, N_GROUP], F32, tag="gneg")
                nc.vector.tensor_scalar(
                    gneg[:], gsum[:], g8[:, TOPK_GROUP - 1:TOPK_GROUP], NEG,
                    op0=OP.is_lt, op1=OP.mult,
                )

                # ---- masked biased scores; top-8 experts ----
                tmp = wpool.tile([P, E], F32, tag="tmp")
                tmp3 = tmp[:].rearrange("p (g e) -> p g e", g=N_GROUP)
                nc.vector.tensor_tensor(
                    tmp3, sfc3, gneg[:, :, None].to_broadcast([P, N_GROUP, EG]),
                    op=OP.add,
                )
                max8 = wpool.tile([P, 8], F32, tag="max8")
                nc.vector.max(max8[:], tmp[:])
                idx8 = idx_all[:, j, :]
                nc.vector.max_index(idx8, max8[:], tmp[:])

                # ---- selected-set mask via match_replace diff ----
                zap = wpool.tile([P, E], F32, tag="zap")
                nc.vector.match_replace(
                    zap[:], in_to_replace=max8[:], in_values=tmp[:], imm_value=NEG
                )
                # sm = scores where selected else ~NEG (Pool rejects
                # comparison ALU ops, so these stay on DVE):
                # q = (tmp == zap)  [1 at non-selected];  sm = q*NEG + scores
                q = wpool.tile([P, E], F32, tag="q")
                qstt = nc.vector.scalar_tensor_tensor(
                    q[:], tmp[:], 1.0, zap[:], op0=OP.mult, op1=OP.is_equal,
                )
                _reserve(nc, nc.vector, qstt, 2, prev=None)
                sm = wpool.tile([P, E], F32, tag="sm")
                smstt = nc.vector.scalar_tensor_tensor(
                    sm[:], q[:], NEG, scores[:], op0=OP.mult, op1=OP.add,
                )
                _reserve(nc, nc.vector, smstt, 2, prev=None)
                prev_gp = badd

                # ---- unbiased scores of the selected 8, sorted by score ----
                smax8 = wpool.tile([P, 8], F32, tag="smax8")
                nc.vector.max(smax8[:], sm[:])
                sidx8 = wpool.tile([P, 8], U32, tag="sidx8")
                nc.vector.max_index(sidx8[:], smax8[:], sm[:])

                # ---- reorder scores to biased-rank order: w8[k] = sum_j
                #      smax8[j] * (sidx8[j] == idx8[k]) ----
                idxf = wpool.tile([P, 8], F32, tag="idxf")
                nc.vector.tensor_copy(idxf[:], idx8)
                sidxf = wpool.tile([P, 8], F32, tag="sidxf")
                nc.vector.tensor_copy(sidxf[:], sidx8[:])
                eq = wpool.tile([P, 8, 8], F32, tag="eq")
                nc.vector.tensor_tensor(
                    eq[:],
                    idxf[:, :, None].to_broadcast([P, 8, 8]),
                    sidxf[:, None, :].to_broadcast([P, 8, 8]),
                    op=OP.is_equal,
                )
                wprod = wpool.tile([P, 8, 8], F32, tag="wprod")
                nc.vector.tensor_tensor(
                    wprod[:], eq[:], smax8[:, None, :].to_broadcast([P, 8, 8]),
                    op=OP.mult,
                )
                w8 = wpool.tile([P, 8], F32, tag="w8")
                nc.vector.reduce_sum(w8[:], wprod[:], axis=mybir.AxisListType.X)

                # ---- normalize: w = 2.5 * w / (sum(w) + 1e-20) ----
                den = wpool.tile([P, 1], F32, tag="den")
                nc.vector.reduce_sum(den[:], w8[:], axis=mybir.AxisListType.X)
                nc.vector.tensor_scalar_add(den[:], den[:], 1e-20)
                rden = wpool.tile([P, 1], F32, tag="rden")
                nc.vector.reciprocal(rden[:], den[:])
                prev_dve = nc.vector.tensor_scalar(
                    w_all[:, j, :], w8[:], rden[:], ROUTED_SCALING,
                    op0=OP.mult, op1=OP.mult,
                )

            d1 = nc.sync.dma_start(idx3, idx_all[:])
            _reserve(nc, nc.sync, d1, 2, prev=prev_dma)
            d2 = nc.sync.dma_start(w3, w_all[:])
            _reserve(nc, nc.sync, d2, 2, prev=d1)
            # Tail carriers: Tile's kernel-tail drain on SP waits on every
            # DMA queue sem; give the legalizer enough nops.
            tail = d2.ins
            for _ in range(14):
                nop = nc.sync.nop(nofuse=True)
                add_dep_helper(nop.ins, tail, sync=False,
                               reason="tail drain wait carriers")
                tail = nop.ins

    _legalize_waits(nc)
    return nc


class _Runner:
    """Compile-once SPMD runner (mirrors bass2jax.run_bass_via_pjrt's
    multi-core path, but holds the jitted fn so repeated calls don't
    re-trace/re-jit; inputs can stay resident on device for timing)."""

    def __init__(self, nc):
        import jax
        from jax.experimental.shard_map import shard_map
        from jax.sharding import Mesh, NamedSharding, PartitionSpec

        from concourse import bass2jax

        bass2jax.install_neuronx_cc_hook()
        self._jax = jax
        self.nc = nc

        partition_name = (
            nc.partition_id_tensor.name if nc.partition_id_tensor else None
        )
        in_names, out_names, out_avals, zero_outs = [], [], [], []
        for alloc in nc.m.functions[0].allocations:
            if not isinstance(alloc, mybir.MemoryLocationSet):
                continue
            name = alloc.memorylocations[0].name
            if alloc.kind == "ExternalInput":
                if name != partition_name:
                    in_names.append(name)
            elif alloc.kind == "ExternalOutput":
                shape = tuple(alloc.tensor_shape)
                dtype = mybir.dt.np(alloc.dtype)
                out_names.append(name)
                out_avals.append(jax.core.ShapedArray(shape, dtype))
                zero_outs.append(np.zeros(shape, dtype))
        self.in_names = list(in_names)
        self.out_names = out_names
        self.out_avals = out_avals
        self.zero_outs = zero_outs
        n_params = len(in_names)
        self.n_params = n_params

        all_names = in_names + out_names
        if partition_name is not None:
            all_names.append(partition_name)

        def _body(*args):
            operands = list(args)
            if partition_name is not None:
                operands.append(bass2jax.partition_id_tensor())
            outs = bass2jax._bass_exec_p.bind(
                *operands,
                out_avals=tuple(out_avals),
                in_names=tuple(all_names),
                out_names=tuple(out_names),
                lowering_input_output_aliases=(),
                sim_require_finite=True,
                sim_require_nnan=True,
                nc=nc,
            )
            return tuple(outs)

        devices = jax.devices()[:N_CORES]
        assert len(devices) == N_CORES
        self.mesh = Mesh(np.asarray(devices), ("core",))
        n_outs = len(out_names)
        in_specs = (PartitionSpec("core"),) * (n_params + n_outs)
        out_specs = (PartitionSpec("core"),) * n_outs
        donate = tuple(range(n_params, n_params + n_outs))
        self._fn = jax.jit(
            shard_map(
                _body, mesh=self.mesh, in_specs=in_specs, out_specs=out_specs,
                check_rep=False,
            ),
            donate_argnums=donate,
            keep_unused=True,
        )
        self._sharding = NamedSharding(self.mesh, PartitionSpec("core"))

    def put_inputs(self, in_maps):
        """Concat per-core inputs on axis 0 and move to device once."""
        concat = [
            np.concatenate([np.asarray(m[name]) for m in in_maps], axis=0)
            for name in self.in_names
        ]
        return [self._jax.device_put(a, self._sharding) for a in concat]

    def _zeros(self):
        return [
            np.zeros((N_CORES * z.shape[0], *z.shape[1:]), z.dtype)
            for z in self.zero_outs
        ]

    def execute(self, dev_inputs):
        outs = self._fn(*dev_inputs, *self._zeros())
        self._jax.block_until_ready(outs)
        return outs

    def run(self, in_maps):
        dev_inputs = self.put_inputs(in_maps)
        out_arrs = self.execute(dev_inputs)
        return [
            {
                name: np.asarray(out_arrs[i]).reshape(
                    N_CORES, *self.out_avals[i].shape
                )[c]
                for i, name in enumerate(self.out_names)
            }
            for c in range(N_CORES)
        ]


_RUNNER_CACHE = {}


def _get_runner():
    if "nc" not in _RUNNER_CACHE:
        _RUNNER_CACHE["nc"] = _Runner(build_nc())
    return _RUNNER_CACHE["nc"]


def make_in_maps(hidden_states, weight, e_score_correction_bias):
    x = np.ascontiguousarray(np.asarray(hidden_states), dtype=np.float32)
    x = x.reshape(T_FULL, H)
    w = np.asarray(weight, dtype=np.float32)
    b = np.asarray(e_score_correction_bias, dtype=np.float32)

    # Split: v = hi + lo with hi = fp16(v), lo = v - hi (exact in f32).
    x_hi = x.astype(F16_NP)
    x_lo = x - x_hi.astype(np.float32)
    w_hi = w.astype(F16_NP)
    w_lo = w - w_hi.astype(np.float32)

    def pack_x(xs):
        # [2048, 4096] -> [j, t, c, p] -> [j, p, c, t] -> [(j p), (c t)]
        return np.ascontiguousarray(
            xs.reshape(TOK_TILES, P, KC, P).transpose(0, 3, 2, 1)
        ).reshape(TOK_TILES * P, KC * P)

    def pack_w(ws):
        # [256, 4096] f32 -> [c, p, e] -> [p, c, e]
        return np.ascontiguousarray(
            ws.T.reshape(KC, P, E).transpose(1, 0, 2)
        )

    wh16 = pack_w(w_hi.astype(np.float32)).astype(F16_NP).reshape(P, KC * E)
    wc8 = np.concatenate(
        [pack_w(w_hi.astype(np.float32) * S2), pack_w(w_lo * S4)], axis=1
    ).astype(F8_NP).reshape(P, 2 * KC * E)
    biasb = np.ascontiguousarray(np.broadcast_to(b, (P, E)))

    in_maps = []
    for i in range(N_CORES):
        sl = slice(i * T_CORE, (i + 1) * T_CORE)
        in_maps.append({
            "xh16": pack_x(x_hi[sl].astype(np.float32)).astype(F16_NP),
            "xl8": pack_x(x_lo[sl] * S1).astype(F8_NP),
            "xh8": pack_x(x_hi[sl].astype(np.float32) * S3).astype(F8_NP),
            "wh16": wh16,
            "wc8": wc8,
            "biasb": biasb,
        })
    return in_maps


def kernel(hidden_states, weight, e_score_correction_bias):
    runner = _get_runner()
    results = runner.run(
        make_in_maps(hidden_states, weight, e_score_correction_bias)
    )
    topk_idx = np.concatenate(
        [r["idx_out"].astype(np.int32) for r in results], axis=0
    )
    topk_weight = np.concatenate([r["w_out"] for r in results], axis=0)
    return topk_idx, topk_weight


# revision 20
# speedup vs baseline: 476.8618x; 1.0084x over previous
"""MiMo V2 MoE gate (sigmoid routing, grouped top-k) on 8 Trainium2 cores.

Contract: kernel(**inputs) takes the FULL unsharded inputs
(hidden_states [4,4096,4096] f32, weight [256,4096] f32,
e_score_correction_bias [256] f32) and returns (topk_idx int32 [16384,8],
topk_weight f32 [16384,8]) matching reference.py.

Strategy (data-parallel over tokens, 2048/core):
  - The gate GEMM runs as an exact-enough split product (measured
    7/131072 idx flips, e_idx 4.8e-3):
        logits = xh@wh  +  2^-16 * (s1*xl @ s2*wh  +  s3*xh @ s4*wl)
    with xh=fp16(x), xl=x-xh, wh=fp16(w), wl=w-wh. The main product is
    one fp16 matmul pass; the two correction products run as fp8e4m3
    DoubleRow matmuls (2 contraction chunks per instruction, 2x ALU),
    with power-of-2 scales s1*s2 == s3*s4 == 2^16 chosen to center each
    operand in fp8 range. Main and correction interleave per chunk so
    the PE's LDWEIGHTS reorder window hides the DoubleRow weight-load
    tax behind fp16 matmul streaming.
  - Host pre-packs every x operand into per-tile PE layout
    [tile, 128 h-part, chunk, 128 tok] so each DMA partition line is one
    contiguous run (strided loads ran 512B descriptors at ~40% of HBM
    rate).
  - Per tile: PSUM accumulates main [128,256] and correction [128,256];
    GpSimd fuses them (psc*2^-16 + ps), ScalarE applies sigmoid, and the
    grouped top-k runs on VectorE via DVE sort8 primitives
    (max / max_index / match_replace), with the bias-add and
    selected-score masking offloaded to GpSimd. The topk weights
    (scores at the selected experts, ordered by biased-score rank) are
    recovered without a gather via an 8x8 index-match between the two
    sort orders.
"""

import sys

if "/opt/trn_rl_repo" not in sys.path:
    sys.path.insert(0, "/opt/trn_rl_repo")

import ml_dtypes
import numpy as np

import concourse.bass as bass
import concourse.mybir as mybir
import concourse.tile as tile
from concourse.tile_rust import add_dep_helper, annotate_deps

P = 128
H = 4096
E = 256
N_CORES = 8
T_FULL = 16384
T_CORE = T_FULL // N_CORES  # 2048
KC = H // P                 # 32 contraction chunks
TOK_TILES = T_CORE // P     # 16 token tiles per core
N_GROUP = 8
EG = E // N_GROUP           # 32 experts per group
TOPK_GROUP = 4
TOP_K = 8
ROUTED_SCALING = 2.5
NEG = -1e30

F32 = mybir.dt.float32
F16 = mybir.dt.float16
F8 = mybir.dt.float8e4
U32 = mybir.dt.uint32
AF = mybir.ActivationFunctionType
OP = mybir.AluOpType

F16_NP = np.float16
F8_NP = ml_dtypes.float8_e4m3

# Correction scales: corr_psum = S * (xl@wh + xh@wl), S = 2^16.
S1 = 2.0 ** 11   # xl pre-scale
S2 = 2.0 ** 5    # wh pre-scale (pairs with xl)
S3 = 2.0 ** 0    # xh pre-scale
S4 = 2.0 ** 16   # wl pre-scale (pairs with xh)
CORR_SCALE = 2.0 ** -16


def _reserve(nc, eng, X, n, prev=None):
    """Emit n plain nops on X's engine, ordered after `prev` (a
    BassInstruction or None) and before X. They act as spare 1-wait
    carriers for _legalize_waits (every TPB instruction has exactly one
    HW wait slot; Tile can assign several waits to one instruction,
    which walrus then rejects)."""
    last = prev.ins if prev is not None else None
    for _ in range(n):
        nop = eng.nop(nofuse=True)
        if last is not None:
            add_dep_helper(nop.ins, last, sync=False,
                           reason="chain reserve nop after predecessor")
        add_dep_helper(X.ins, nop.ins, sync=False,
                       reason="reserve nop precedes its instruction")
        last = nop.ins


def _legalize_waits(nc, report=None):
    """Every TPB instruction has ONE hardware wait slot; Tile can assign
    several on_wait entries to an instruction, which walrus rejects
    ("Too many sync wait commands"). Fix in two ways, per engine stream
    (scheduled order):
      1. value-floor dedup: drop waits already implied by an earlier wait
         on the same semaphore in this stream (monotonic sems).
      2. excess-wait hoisting: move extra waits onto the nearest earlier
         wait-free instruction, scanning only across instructions with no
         on_update (pure nops) -- crossing an updater could reorder a
         producer chain and deadlock; this rule keeps placements provably
         safe. _reserve() plants such nops next to risky instructions.
    Drains are skipped (they encode multi-sem waits natively)."""
    leftover = []
    if True:
        # The kernel CFG is linear (main block -> end block), so per-engine
        # program order is the block-order concatenation.
        nonmono = set()  # sems that ever decrease (barrier sems): no
                         # floor-dedup for their waits
        for blk in nc.m.functions[0].blocks:
            for inst in blk.instructions:
                si = inst.sync_info
                for u in (si.on_update if si and si.on_update else []):
                    if str(u.update_mode) not in ('sem-inc', 'sem-add-imm'):
                        nonmono.add(u.id)
        # Pass 1: per-engine value-floor dedup of monotonic sem-ge waits.
        floors = {}
        for blk in nc.m.functions[0].blocks:
            for X in blk.instructions:
                si = X.sync_info
                if si is None or not si.on_wait:
                    continue
                floor = floors.setdefault(str(X.engine), {})
                mode_ok = lambda w: (str(w.wait_mode) == 'sem-ge-imm'
                                     and w.id not in nonmono)
                waits = []
                for w in si.on_wait:
                    if (mode_ok(w) and w.id in floor
                            and floor[w.id] >= w.wait_value):
                        continue  # already implied earlier in this stream
                    waits.append(w)
                    if mode_ok(w):
                        floor[w.id] = max(floor.get(w.id, 0), w.wait_value)
                X.sync_info = mybir.SyncInfo(
                    on_wait=waits,
                    on_update=list(si.on_update) if si.on_update else [])
        # Pass 2: any instruction still holding >1 waits gets all but one
        # moved onto fresh carrier nops inserted DIRECTLY before it in the
        # block (post-scheduling, so adjacency is guaranteed and the waits
        # execute at the same engine-stream position — semantically
        # identical to multiple waits on one instruction). This includes
        # the kernel-tail drains: their native multi-sem budget overflows
        # with this many DMA queues.
        carrier_id = [0]
        for blk in nc.m.functions[0].blocks:
            inserts = []
            for i, X in enumerate(blk.instructions):
                si = X.sync_info
                if si is None or not si.on_wait or len(si.on_wait) <= 1:
                    continue
                waits = list(si.on_wait)
                extra, keep = waits[:-1], waits[-1:]
                nops = []
                for w in extra:
                    nop = mybir.InstNoOp(
                        name=f"LW-{carrier_id[0]}", ins=[], outs=[])
                    carrier_id[0] += 1
                    nop.engine = X.engine
                    nop.bass_nofuse = True
                    nop.sync_info = mybir.SyncInfo(on_wait=[w], on_update=[])
                    nops.append(nop)
                inserts.append((i, nops))
                X.sync_info = mybir.SyncInfo(
                    on_wait=keep,
                    on_update=list(si.on_update) if si.on_update else [])
            for i, nops in reversed(inserts):
                blk.instructions[i:i] = nops
    # The PE gate ENGINE_NOPs carry AP operands purely for Tile dep
    # tracking; walrus's engine check rejects a nop with operands, so
    # strip them now (tile.py does the same for InstNoOp instructions).
    for blk in nc.m.functions[0].blocks:
        for inst in blk.instructions:
            if (isinstance(inst, mybir.InstISA) and (inst.ins or inst.outs)
                    and inst.op_name == 'ENGINE_NOP'):
                inst.ins = []
                inst.outs = []

    if report is not None:
        report.extend(leftover)
    elif leftover:
        raise RuntimeError(f"wait legalization failed for: {leftover}")


def build_nc():
    nc = bass.Bass()

    # Host-packed per-tile x layouts: [(j p), (c t)] so tile j is a
    # [128, KC*128] slab with one contiguous line per partition.
    xh16 = nc.dram_tensor("xh16", [TOK_TILES * P, KC * P], F16, kind="ExternalInput")
    xl8 = nc.dram_tensor("xl8", [TOK_TILES * P, KC * P], F8, kind="ExternalInput")
    xh8 = nc.dram_tensor("xh8", [TOK_TILES * P, KC * P], F8, kind="ExternalInput")
    # Host-packed weights: [p, (c e)].
    wh16 = nc.dram_tensor("wh16", [P, KC * E], F16, kind="ExternalInput")
    # wc8 carries 2*KC chunks: chunks 0..31 = s2*wh, 32..63 = s4*wl.
    wc8 = nc.dram_tensor("wc8", [P, 2 * KC * E], F8, kind="ExternalInput")
    biasb = nc.dram_tensor("biasb", [P, E], F32, kind="ExternalInput")
    idx_out = nc.dram_tensor("idx_out", [T_CORE, TOP_K], U32, kind="ExternalOutput")
    w_out = nc.dram_tensor("w_out", [T_CORE, TOP_K], F32, kind="ExternalOutput")

    xh16_4 = xh16.ap().rearrange("(j p) (c t) -> p j c t", p=P, c=KC)
    xl8_4 = xl8.ap().rearrange("(j p) (c t) -> p j c t", p=P, c=KC)
    xh8_4 = xh8.ap().rearrange("(j p) (c t) -> p j c t", p=P, c=KC)
    wh16_3 = wh16.ap().rearrange("p (c e) -> p c e", c=KC)
    wc8_3 = wc8.ap().rearrange("p (c e) -> p c e", c=2 * KC)
    idx3 = idx_out.ap().rearrange("(j p) k -> p j k", p=P)  # [128, 16, 8]
    w3 = w_out.ap().rearrange("(j p) k -> p j k", p=P)

    with tile.TileContext(nc) as tc:
        with (
            tc.tile_pool(name="const", bufs=1) as cpool,
            tc.tile_pool(name="xhin", bufs=8) as xhpool,
            tc.tile_pool(name="xl8in", bufs=6) as xl8pool,
            tc.tile_pool(name="xh8in", bufs=6) as xh8pool,
            # 4 PSUM buffers per tag (2 banks of 8): the post-GEMM chain is
            # ~7us deep, so 2 buffers stalled the PE ~2.4us/tile waiting
            # for slot release.
            tc.tile_pool(name="psum", bufs=4, space="PSUM") as pspool,
            tc.tile_pool(name="work", bufs=3) as wpool,
        ):
            # DMA order: wh16 then tile-0 x parts before the big wc8 so
            # tile 0's matmuls can start as early as possible.
            wh = cpool.tile([P, KC, E], F16)
            nc.sync.dma_start(wh[:], wh16_3)
            wc = cpool.tile([P, 2 * KC, E], F8)
            bsb = cpool.tile([P, E], F32)
            idx_all = cpool.tile([P, TOK_TILES, TOP_K], U32)
            w_all = cpool.tile([P, TOK_TILES, TOP_K], F32)

            prev_sig = None
            prev_mm = None
            prev_dma = None
            prev_gp = None
            prev_dve = None
            for j in range(TOK_TILES):
                xh = xhpool.tile([P, KC, P], F16, tag="xh")
                xh_dma = nc.sync.dma_start(xh[:], xh16_4[:, j])
                _reserve(nc, nc.sync, xh_dma, 3, prev=prev_dma)
                xl = xl8pool.tile([P, KC, P], F8, tag="xl")
                xl_dma = nc.sync.dma_start(xl[:], xl8_4[:, j])
                _reserve(nc, nc.sync, xl_dma, 3, prev=xh_dma)
                x8 = xh8pool.tile([P, KC, P], F8, tag="x8")
                x8_dma = nc.sync.dma_start(x8[:], xh8_4[:, j])
                _reserve(nc, nc.sync, x8_dma, 3, prev=xl_dma)
                prev_dma = x8_dma
                if j == 0:
                    wc_dma = nc.sync.dma_start(wc[:], wc8_3)
                    _reserve(nc, nc.sync, wc_dma, 2, prev=prev_dma)
                    b_dma = nc.sync.dma_start(bsb[:], biasb.ap())
                    _reserve(nc, nc.sync, b_dma, 2, prev=wc_dma)
                    prev_dma = b_dma

                ps = pspool.tile([P, E], F32, tag="ps")
                psc = pspool.tile([P, E], F32, tag="psc")
                # The fused matmul (self-loading LDWEIGHTS) only has budget
                # for ONE semaphore wait in walrus codegen, but the
                # tile-leading matmul needs the x-DMA sems plus the
                # psum-slot-release sems. Emit a PE NoOp that declares those
                # data deps (1-elem APs, registered via annotate_deps) so
                # Tile's per-engine clock absorbs all waits there; the
                # matmuls then follow wait-free in PE program order. Tile
                # strips APs from InstNoOp at lowering, so walrus only
                # sees a plain NOP.
                gate = nc.tensor.nop(nofuse=True)
                gate.ins.ins = [nc.tensor.lower_ap(xh[0:1, 0, 0:1])]
                gate.ins.outs = [nc.tensor.lower_ap(ps[0:1, 0:1])]
                annotate_deps(tc.dep_state, gate.ins, tc.shadow_memory,
                              tc._rust_ctx, nc.inst_map)
                _reserve(nc, nc.tensor, gate, 4, prev=prev_mm)
                # Main fp16 pass (one contiguous accumulation group —
                # interleaving two groups miscompiles; the PE's 64-deep
                # reorder window still lets the correction group's
                # LDWEIGHTS creep into this stream).
                for i in range(KC):
                    nc.tensor.matmul(
                        ps[:], lhsT=xh[:, i, :], rhs=wh[:, i, :],
                        start=(i == 0), stop=(i == KC - 1),
                    )
                # Correction fp8 DoubleRow pass: pair i contracts chunks
                # (2i, 2i+1); pairs 0..15 are the xl-block, 16..31 the
                # xh-block.
                gate2 = nc.tensor.nop(nofuse=True)
                gate2.ins.ins = [
                    nc.tensor.lower_ap(xl[0:1, 0, 0:1]),
                    nc.tensor.lower_ap(x8[0:1, 0, 0:1]),
                ]
                gate2.ins.outs = [nc.tensor.lower_ap(psc[0:1, 0:1])]
                annotate_deps(tc.dep_state, gate2.ins, tc.shadow_memory,
                              tc._rust_ctx, nc.inst_map)
                _reserve(nc, nc.tensor, gate2, 4, prev=gate)
                for i in range(KC):
                    if i < KC // 2:
                        clhs, coff = xl, 2 * i
                    else:
                        clhs, coff = x8, 2 * (i - KC // 2)
                    mm = nc.tensor.matmul(
                        psc[:],
                        lhsT=clhs[:, coff:coff + 2, :],
                        rhs=wc[:, 2 * i:2 * i + 2, :],
                        start=(i == 0), stop=(i == KC - 1),
                        perf_mode=mybir.MatmulPerfMode.DoubleRow,
                    )
                prev_mm = mm

                # ---- logits = ps + 2^-16 * psc, sigmoid ----
                # Engines may read only ONE input from PSUM per instruction
                # (and GpSimd none), so: ACT scaled-copies psc to SBUF
                # (Copy bypasses the activation table, so no table thrash
                # with Sigmoid), then DVE adds ps (PSUM) + cb (SBUF).
                cb = wpool.tile([P, E], F32, tag="cb")
                cp = nc.scalar.activation(cb[:], psc[:], AF.Copy,
                                          scale=CORR_SCALE)
                _reserve(nc, nc.scalar, cp, 3, prev=prev_sig)
                u = wpool.tile([P, E], F32, tag="u")
                stt = nc.vector.tensor_add(u[:], ps[:], cb[:])
                _reserve(nc, nc.vector, stt, 3, prev=prev_dve)
                scores = wpool.tile([P, E], F32, tag="scores")
                sig = nc.scalar.activation(scores[:], u[:], AF.Sigmoid)
                _reserve(nc, nc.scalar, sig, 2, prev=cp)
                prev_sig = sig
                sfc = wpool.tile([P, E], F32, tag="sfc")
                badd = nc.gpsimd.tensor_add(sfc[:], scores[:], bsb[:])
                _reserve(nc, nc.gpsimd, badd, 2, prev=prev_gp)
                sfc3 = sfc[:].rearrange("p (g e) -> p g e", g=N_GROUP)

                # ---- group scores: sum of top-2 per group of 32 ----
                # max1 = per-group max; zap first occurrence of each (the 8
                # group maxes are bitwise-distinct for random data); max2 =
                # per-group max of the rest; gsum = max1 + max2.
                gmax1 = wpool.tile([P, N_GROUP], F32, tag="gmax1")
                nc.vector.reduce_max(gmax1[:], sfc3, axis=mybir.AxisListType.X)
                zapg = wpool.tile([P, E], F32, tag="zapg")
                nc.vector.match_replace(
                    zapg[:], in_to_replace=gmax1[:], in_values=sfc[:],
                    imm_value=NEG,
                )
                zapg3 = zapg[:].rearrange("p (g e) -> p g e", g=N_GROUP)
                gsum = wpool.tile([P, N_GROUP], F32, tag="gsum")
                nc.vector.reduce_max(gsum[:], zapg3, axis=mybir.AxisListType.X)
                nc.vector.tensor_add(gsum[:], gsum[:], gmax1[:])

                # ---- pick top-4 groups; additive mask 0 / -BIG ----
                g8 = wpool.tile([P, 8], F32, tag="g8")
                nc.vector.max(g8[:], gsum[:])
                gneg = wpool.tile([P, N_GROUP], F32, tag="gneg")
                nc.vector.tensor_scalar(
                    gneg[:], gsum[:], g8[:, TOPK_GROUP - 1:TOPK_GROUP], NEG,
                    op0=OP.is_lt, op1=OP.mult,
                )

                # ---- masked biased scores; top-8 experts ----
                tmp = wpool.tile([P, E], F32, tag="tmp")
                tmp3 = tmp[:].rearrange("p (g e) -> p g e", g=N_GROUP)
                nc.vector.tensor_tensor(
                    tmp3, sfc3, gneg[:, :, None].to_broadcast([P, N_GROUP, EG]),
                    op=OP.add,
                )
                max8 = wpool.tile([P, 8], F32, tag="max8")
                nc.vector.max(max8[:], tmp[:])
                idx8 = idx_all[:, j, :]
                nc.vector.max_index(idx8, max8[:], tmp[:])

                # ---- selected-set mask via match_replace diff ----
                zap = wpool.tile([P, E], F32, tag="zap")
                nc.vector.match_replace(
                    zap[:], in_to_replace=max8[:], in_values=tmp[:], imm_value=NEG
                )
                # sm = scores where selected else ~NEG (Pool rejects
                # comparison ALU ops, so these stay on DVE):
                # q = (tmp == zap)  [1 at non-selected];  sm = q*NEG + scores
                q = wpool.tile([P, E], F32, tag="q")
                qstt = nc.vector.scalar_tensor_tensor(
                    q[:], tmp[:], 1.0, zap[:], op0=OP.mult, op1=OP.is_equal,
                )
                _reserve(nc, nc.vector, qstt, 2, prev=None)
                sm = wpool.tile([P, E], F32, tag="sm")
                smstt = nc.vector.scalar_tensor_tensor(
                    sm[:], q[:], NEG, scores[:], op0=OP.mult, op1=OP.add,
                )
                _reserve(nc, nc.vector, smstt, 2, prev=None)
                prev_gp = badd

                # ---- unbiased scores of the selected 8, sorted by score ----
                smax8 = wpool.tile([P, 8], F32, tag="smax8")
                nc.vector.max(smax8[:], sm[:])
                sidx8 = wpool.tile([P, 8], U32, tag="sidx8")
                nc.vector.max_index(sidx8[:], smax8[:], sm[:])

                # ---- reorder scores to biased-rank order: w8[k] = sum_j
                #      smax8[j] * (sidx8[j] == idx8[k]) ----
                idxf = wpool.tile([P, 8], F32, tag="idxf")
                nc.vector.tensor_copy(idxf[:], idx8)
                sidxf = wpool.tile([P, 8], F32, tag="sidxf")
                nc.vector.tensor_copy(sidxf[:], sidx8[:])
                eq = wpool.tile([P, 8, 8], F32, tag="eq")
                nc.vector.tensor_tensor(
                    eq[:],
                    idxf[:, :, None].to_broadcast([P, 8, 8]),
                    sidxf[:, None, :].to_broadcast([P, 8, 8]),
                    op=OP.is_equal,
                )
                wprod = wpool.tile([P, 8, 8], F32, tag="wprod")
                nc.vector.tensor_tensor(
                    wprod[:], eq[:], smax8[:, None, :].to_broadcast([P, 8, 8]),
                    op=OP.mult,
                )
                w8 = wpool.tile([P, 8], F32, tag="w8")
                nc.vector.reduce_sum(w8[:], wprod[:], axis=mybir.AxisListType.X)

                # ---- normalize: w = 2.5 * w / (sum(w) + 1e-20) ----
                den = wpool.tile([P, 1], F32, tag="den")
                nc.vector.reduce_sum(den[:], w8[:], axis=mybir.AxisListType.X)
                nc.vector.tensor_scalar_add(den[:], den[:], 1e-20)
                rden = wpool.tile([P, 1], F32, tag="rden")
                nc.vector.reciprocal(rden[:], den[:])
                prev_dve = nc.vector.tensor_scalar(
                    w_all[:, j, :], w8[:], rden[:], ROUTED_SCALING,
                    op0=OP.mult, op1=OP.mult,
                )

            d1 = nc.sync.dma_start(idx3, idx_all[:])
            _reserve(nc, nc.sync, d1, 2, prev=prev_dma)
            d2 = nc.sync.dma_start(w3, w_all[:])
            _reserve(nc, nc.sync, d2, 2, prev=d1)
            # Tail carriers: Tile's kernel-tail drain on SP waits on every
            # DMA queue sem; give the legalizer enough nops.
            tail = d2.ins
            for _ in range(14):
                nop = nc.sync.nop(nofuse=True)
                add_dep_helper(nop.ins, tail, sync=False,
                               reason="tail drain wait carriers")
                tail = nop.ins

    _legalize_waits(nc)
    return nc


class _Runner:
    """Compile-once SPMD runner (mirrors bass2jax.run_bass_via_pjrt's
    multi-core path, but holds the jitted fn so repeated calls don't
    re-trace/re-jit; inputs can stay resident on device for timing)."""

    def __init__(self, nc):
        import jax
        from jax.experimental.shard_map import shard_map
        from jax.sharding import Mesh, NamedSharding, PartitionSpec

        from concourse import bass2jax

        bass2jax.install_neuronx_cc_hook()
        self._jax = jax
        self.nc = nc

        partition_name = (
            nc.partition_id_tensor.name if nc.partition_id_tensor else None
        )
        in_names, out_names, out_avals, zero_outs = [], [], [], []
        for alloc in nc.m.functions[0].allocations:
            if not isinstance(alloc, mybir.MemoryLocationSet):
                continue
            name = alloc.memorylocations[0].name
            if alloc.kind == "ExternalInput":
                if name != partition_name:
                    in_names.append(name)
            elif alloc.kind == "ExternalOutput":
                shape = tuple(alloc.tensor_shape)
                dtype = mybir.dt.np(alloc.dtype)
                out_names.append(name)
                out_avals.append(jax.core.ShapedArray(shape, dtype))
                zero_outs.append(np.zeros(shape, dtype))
        self.in_names = list(in_names)
        self.out_names = out_names
        self.out_avals = out_avals
        self.zero_outs = zero_outs
        n_params = len(in_names)
        self.n_params = n_params

        all_names = in_names + out_names
        if partition_name is not None:
            all_names.append(partition_name)

        def _body(*args):
            operands = list(args)
            if partition_name is not None:
                operands.append(bass2jax.partition_id_tensor())
            outs = bass2jax._bass_exec_p.bind(
                *operands,
                out_avals=tuple(out_avals),
                in_names=tuple(all_names),
                out_names=tuple(out_names),
                lowering_input_output_aliases=(),
                sim_require_finite=True,
                sim_require_nnan=True,
                nc=nc,
            )
            return tuple(outs)

        devices = jax.devices()[:N_CORES]
        assert len(devices) == N_CORES
        self.mesh = Mesh(np.asarray(devices), ("core",))
        n_outs = len(out_names)
        in_specs = (PartitionSpec("core"),) * (n_params + n_outs)
        out_specs = (PartitionSpec("core"),) * n_outs
        donate = tuple(range(n_params, n_params + n_outs))
        self._fn = jax.jit(
            shard_map(
                _body, mesh=self.mesh, in_specs=in_specs, out_specs=out_specs,
                check_rep=False,
            ),
            donate_argnums=donate,
            keep_unused=True,
        )
        self._sharding = NamedSharding(self.mesh, PartitionSpec("core"))

    def put_inputs(self, in_maps):
        """Concat per-core inputs on axis 0 and move to device once."""
        concat = [
            np.concatenate([np.asarray(m[name]) for m in in_maps], axis=0)
            for name in self.in_names
        ]
        return [self._jax.device_put(a, self._sharding) for a in concat]

    def _zeros(self):
        return [
            np.zeros((N_CORES * z.shape[0], *z.shape[1:]), z.dtype)
            for z in self.zero_outs
        ]

    def execute(self, dev_inputs):
        outs = self._fn(*dev_inputs, *self._zeros())
        self._jax.block_until_ready(outs)
        return outs

    def run(self, in_maps):
        dev_inputs = self.put_inputs(in_maps)
        out_arrs = self.execute(dev_inputs)
        return [
            {
                name: np.asarray(out_arrs[i]).reshape(
                    N_CORES, *self.out_avals[i].shape
                )[c]
                for i, name in enumerate(self.out_names)
            }
            for c in range(N_CORES)
        ]


_RUNNER_CACHE = {}


def _get_runner():
    if "nc" not in _RUNNER_CACHE:
        _RUNNER_CACHE["nc"] = _Runner(build_nc())
    return _RUNNER_CACHE["nc"]


def make_in_maps(hidden_states, weight, e_score_correction_bias):
    x = np.ascontiguousarray(np.asarray(hidden_states), dtype=np.float32)
    x = x.reshape(T_FULL, H)
    w = np.asarray(weight, dtype=np.float32)
    b = np.asarray(e_score_correction_bias, dtype=np.float32)

    # Split: v = hi + lo with hi = fp16(v), lo = v - hi (exact in f32).
    x_hi = x.astype(F16_NP)
    x_lo = x - x_hi.astype(np.float32)
    w_hi = w.astype(F16_NP)
    w_lo = w - w_hi.astype(np.float32)

    def pack_x(xs):
        # [2048, 4096] -> [j, t, c, p] -> [j, p, c, t] -> [(j p), (c t)]
        return np.ascontiguousarray(
            xs.reshape(TOK_TILES, P, KC, P).transpose(0, 3, 2, 1)
        ).reshape(TOK_TILES * P, KC * P)

    def pack_w(ws):
        # [256, 4096] f32 -> [c, p, e] -> [p, c, e]
        return np.ascontiguousarray(
            ws.T.reshape(KC, P, E).transpose(1, 0, 2)
        )

    wh16 = pack_w(w_hi.astype(np.float32)).astype(F16_NP).reshape(P, KC * E)
    wc8 = np.concatenate(
        [pack_w(w_hi.astype(np.float32) * S2), pack_w(w_lo * S4)], axis=1
    ).astype(F8_NP).reshape(P, 2 * KC * E)
    biasb = np.ascontiguousarray(np.broadcast_to(b, (P, E)))

    in_maps = []
    for i in range(N_CORES):
        sl = slice(i * T_CORE, (i + 1) * T_CORE)
        in_maps.append({
            "xh16": pack_x(x_hi[sl].astype(np.float32)).astype(F16_NP),
            "xl8": pack_x(x_lo[sl] * S1).astype(F8_NP),
            "xh8": pack_x(x_hi[sl].astype(np.float32) * S3).astype(F8_NP),
            "wh16": wh16,
            "wc8": wc8,
            "biasb": biasb,
        })
    return in_maps


def kernel(hidden_states, weight, e_score_correction_bias):
    runner = _get_runner()
    results = runner.run(
        make_in_maps(hidden_states, weight, e_score_correction_bias)
    )
    topk_idx = np.concatenate(
        [r["idx_out"].astype(np.int32) for r in results], axis=0
    )
    topk_weight = np.concatenate([r["w_out"] for r in results], axis=0)
    return topk_idx, topk_weight


# revision 22
# speedup vs baseline: 560.8420x; 1.1761x over previous
"""MiMo V2 MoE gate (sigmoid routing, grouped top-k) on 8 Trainium2 cores.

Contract: kernel(**inputs) takes the FULL unsharded inputs
(hidden_states [4,4096,4096] f32, weight [256,4096] f32,
e_score_correction_bias [256] f32) and returns (topk_idx int32 [16384,8],
topk_weight f32 [16384,8]) matching reference.py.

Strategy (data-parallel over tokens, 2048/core):
  - The gate GEMM runs as an exact-enough split product (measured
    7/131072 idx flips, e_idx 4.8e-3):
        logits = xh@wh  +  2^-16 * (s1*xl @ s2*wh  +  s3*xh @ s4*wl)
    with xh=fp16(x), xl=x-xh, wh=fp16(w), wl=w-wh. The main product is
    one fp16 matmul pass; the two correction products run as fp8e4m3
    DoubleRow matmuls (2 contraction chunks per instruction, 2x ALU),
    with power-of-2 scales s1*s2 == s3*s4 == 2^16 chosen to center each
    operand in fp8 range. Main and correction interleave per chunk so
    the PE's LDWEIGHTS reorder window hides the DoubleRow weight-load
    tax behind fp16 matmul streaming.
  - Host pre-packs every x operand into per-tile PE layout
    [tile, 128 h-part, chunk, 128 tok] so each DMA partition line is one
    contiguous run (strided loads ran 512B descriptors at ~40% of HBM
    rate).
  - Per tile: PSUM accumulates main [128,256] and correction [128,256];
    GpSimd fuses them (psc*2^-16 + ps), ScalarE applies sigmoid, and the
    grouped top-k runs on VectorE via DVE sort8 primitives
    (max / max_index / match_replace), with the bias-add and
    selected-score masking offloaded to GpSimd. The topk weights
    (scores at the selected experts, ordered by biased-score rank) are
    recovered without a gather via an 8x8 index-match between the two
    sort orders.
"""

import sys

if "/opt/trn_rl_repo" not in sys.path:
    sys.path.insert(0, "/opt/trn_rl_repo")

import ml_dtypes
import numpy as np

import concourse.bass as bass
import concourse.mybir as mybir
import concourse.tile as tile
from concourse.tile_rust import add_dep_helper, annotate_deps

P = 128
H = 4096
E = 256
N_CORES = 8
T_FULL = 16384
T_CORE = T_FULL // N_CORES  # 2048
KC = H // P                 # 32 contraction chunks
TOK_TILES = T_CORE // P     # 16 token tiles per core
N_GROUP = 8
EG = E // N_GROUP           # 32 experts per group
TOPK_GROUP = 4
TOP_K = 8
ROUTED_SCALING = 2.5
NEG = -1e30

F32 = mybir.dt.float32
F16 = mybir.dt.float16
F8 = mybir.dt.float8e4
U32 = mybir.dt.uint32
AF = mybir.ActivationFunctionType
OP = mybir.AluOpType

F16_NP = np.float16
F8_NP = ml_dtypes.float8_e4m3

# Correction scales: corr_psum = S * (xl@wh + xh@wl), S = 2^16.
S1 = 2.0 ** 11   # xl pre-scale
S2 = 2.0 ** 5    # wh pre-scale (pairs with xl)
S3 = 2.0 ** 0    # xh pre-scale
S4 = 2.0 ** 16   # wl pre-scale (pairs with xh)
CORR_SCALE = 2.0 ** -16


def _reserve(nc, eng, X, n, prev=None):
    """Emit n plain nops on X's engine, ordered after `prev` (a
    BassInstruction or None) and before X. They act as spare 1-wait
    carriers for _legalize_waits (every TPB instruction has exactly one
    HW wait slot; Tile can assign several waits to one instruction,
    which walrus then rejects)."""
    last = prev.ins if prev is not None else None
    for _ in range(n):
        nop = eng.nop(nofuse=True)
        if last is not None:
            add_dep_helper(nop.ins, last, sync=False,
                           reason="chain reserve nop after predecessor")
        add_dep_helper(X.ins, nop.ins, sync=False,
                       reason="reserve nop precedes its instruction")
        last = nop.ins


def _legalize_waits(nc, report=None):
    """Every TPB instruction has ONE hardware wait slot; Tile can assign
    several on_wait entries to an instruction, which walrus rejects
    ("Too many sync wait commands"). Fix in two ways, per engine stream
    (scheduled order):
      1. value-floor dedup: drop waits already implied by an earlier wait
         on the same semaphore in this stream (monotonic sems).
      2. excess-wait hoisting: move extra waits onto the nearest earlier
         wait-free instruction, scanning only across instructions with no
         on_update (pure nops) -- crossing an updater could reorder a
         producer chain and deadlock; this rule keeps placements provably
         safe. _reserve() plants such nops next to risky instructions.
    Drains are skipped (they encode multi-sem waits natively)."""
    leftover = []
    if True:
        # The kernel CFG is linear (main block -> end block), so per-engine
        # program order is the block-order concatenation.
        nonmono = set()  # sems that ever decrease (barrier sems): no
                         # floor-dedup for their waits
        for blk in nc.m.functions[0].blocks:
            for inst in blk.instructions:
                si = inst.sync_info
                for u in (si.on_update if si and si.on_update else []):
                    if str(u.update_mode) not in ('sem-inc', 'sem-add-imm'):
                        nonmono.add(u.id)
        # Pass 1: per-engine value-floor dedup of monotonic sem-ge waits.
        floors = {}
        for blk in nc.m.functions[0].blocks:
            for X in blk.instructions:
                si = X.sync_info
                if si is None or not si.on_wait:
                    continue
                floor = floors.setdefault(str(X.engine), {})
                mode_ok = lambda w: (str(w.wait_mode) == 'sem-ge-imm'
                                     and w.id not in nonmono)
                waits = []
                for w in si.on_wait:
                    if (mode_ok(w) and w.id in floor
                            and floor[w.id] >= w.wait_value):
                        continue  # already implied earlier in this stream
                    waits.append(w)
                    if mode_ok(w):
                        floor[w.id] = max(floor.get(w.id, 0), w.wait_value)
                X.sync_info = mybir.SyncInfo(
                    on_wait=waits,
                    on_update=list(si.on_update) if si.on_update else [])
        # Pass 2: any instruction still holding >1 waits gets all but one
        # moved onto fresh carrier nops inserted DIRECTLY before it in the
        # block (post-scheduling, so adjacency is guaranteed and the waits
        # execute at the same engine-stream position — semantically
        # identical to multiple waits on one instruction). This includes
        # the kernel-tail drains: their native multi-sem budget overflows
        # with this many DMA queues.
        carrier_id = [0]
        for blk in nc.m.functions[0].blocks:
            inserts = []
            for i, X in enumerate(blk.instructions):
                si = X.sync_info
                if si is None or not si.on_wait or len(si.on_wait) <= 1:
                    continue
                waits = list(si.on_wait)
                extra, keep = waits[:-1], waits[-1:]
                nops = []
                for w in extra:
                    nop = mybir.InstNoOp(
                        name=f"LW-{carrier_id[0]}", ins=[], outs=[])
                    carrier_id[0] += 1
                    nop.engine = X.engine
                    nop.bass_nofuse = True
                    nop.sync_info = mybir.SyncInfo(on_wait=[w], on_update=[])
                    nops.append(nop)
                inserts.append((i, nops))
                X.sync_info = mybir.SyncInfo(
                    on_wait=keep,
                    on_update=list(si.on_update) if si.on_update else [])
            for i, nops in reversed(inserts):
                blk.instructions[i:i] = nops
    # The PE gate ENGINE_NOPs carry AP operands purely for Tile dep
    # tracking; walrus's engine check rejects a nop with operands, so
    # strip them now (tile.py does the same for InstNoOp instructions).
    for blk in nc.m.functions[0].blocks:
        for inst in blk.instructions:
            if (isinstance(inst, mybir.InstISA) and (inst.ins or inst.outs)
                    and inst.op_name == 'ENGINE_NOP'):
                inst.ins = []
                inst.outs = []

    if report is not None:
        report.extend(leftover)
    elif leftover:
        raise RuntimeError(f"wait legalization failed for: {leftover}")


def build_nc():
    nc = bass.Bass()

    # Host-packed per-tile x layouts: [(j p), (c t)] so tile j is a
    # [128, KC*128] slab with one contiguous line per partition.
    xh16 = nc.dram_tensor("xh16", [TOK_TILES * P, KC * P], F16, kind="ExternalInput")
    xl8 = nc.dram_tensor("xl8", [TOK_TILES * P, KC * P], F8, kind="ExternalInput")
    xh8 = nc.dram_tensor("xh8", [TOK_TILES * P, KC * P], F8, kind="ExternalInput")
    # Host-packed weights: [p, (c e)].
    wh16 = nc.dram_tensor("wh16", [P, KC * E], F16, kind="ExternalInput")
    # wc8 carries 2*KC chunks: chunks 0..31 = s2*wh, 32..63 = s4*wl.
    wc8 = nc.dram_tensor("wc8", [P, 2 * KC * E], F8, kind="ExternalInput")
    biasb = nc.dram_tensor("biasb", [P, E], F32, kind="ExternalInput")
    idx_out = nc.dram_tensor("idx_out", [T_CORE, TOP_K], U32, kind="ExternalOutput")
    w_out = nc.dram_tensor("w_out", [T_CORE, TOP_K], F32, kind="ExternalOutput")

    xh16_4 = xh16.ap().rearrange("(j p) (c t) -> p j c t", p=P, c=KC)
    xl8_4 = xl8.ap().rearrange("(j p) (c t) -> p j c t", p=P, c=KC)
    xh8_4 = xh8.ap().rearrange("(j p) (c t) -> p j c t", p=P, c=KC)
    wh16_3 = wh16.ap().rearrange("p (c e) -> p c e", c=KC)
    wc8_3 = wc8.ap().rearrange("p (c e) -> p c e", c=2 * KC)
    idx3 = idx_out.ap().rearrange("(j p) k -> p j k", p=P)  # [128, 16, 8]
    w3 = w_out.ap().rearrange("(j p) k -> p j k", p=P)

    with tile.TileContext(nc) as tc:
        with (
            tc.tile_pool(name="const", bufs=1) as cpool,
            tc.tile_pool(name="xhin", bufs=8) as xhpool,
            tc.tile_pool(name="xl8in", bufs=6) as xl8pool,
            tc.tile_pool(name="xh8in", bufs=6) as xh8pool,
            # 4 PSUM buffers per tag (2 banks of 8): the post-GEMM chain is
            # ~7us deep, so 2 buffers stalled the PE ~2.4us/tile waiting
            # for slot release.
            tc.tile_pool(name="psum", bufs=4, space="PSUM") as pspool,
            tc.tile_pool(name="work", bufs=3) as wpool,
        ):
            # DMA order: wh16 then tile-0 x parts before the big wc8 so
            # tile 0's matmuls can start as early as possible.
            wh = cpool.tile([P, KC, E], F16)
            nc.sync.dma_start(wh[:], wh16_3)
            wc = cpool.tile([P, 2 * KC, E], F8)
            bsb = cpool.tile([P, E], F32)
            idx_all = cpool.tile([P, TOK_TILES, TOP_K], U32)
            w_all = cpool.tile([P, TOK_TILES, TOP_K], F32)

            prev_sig = None
            prev_mm = None
            prev_dma = None
            prev_gp = None
            prev_dve = None
            pending = None

            def emit_topk(j, scores, sfc):
                """Grouped top-k for one 128-token tile (DVE-dominated)."""
                sfc3 = sfc[:].rearrange("p (g e) -> p g e", g=N_GROUP)
                # group scores: sum of top-2 per group of 32. max1 =
                # per-group max; zap first occurrence of each (the 8 group
                # maxes are bitwise-distinct for random data); max2 =
                # per-group max of the rest; gsum = max1 + max2.
                gmax1 = wpool.tile([P, N_GROUP], F32, tag="gmax1")
                nc.vector.reduce_max(gmax1[:], sfc3, axis=mybir.AxisListType.X)
                zapg = wpool.tile([P, E], F32, tag="zapg")
                nc.vector.match_replace(
                    zapg[:], in_to_replace=gmax1[:], in_values=sfc[:],
                    imm_value=NEG,
                )
                zapg3 = zapg[:].rearrange("p (g e) -> p g e", g=N_GROUP)
                gsum = wpool.tile([P, N_GROUP], F32, tag="gsum")
                nc.vector.reduce_max(gsum[:], zapg3, axis=mybir.AxisListType.X)
                nc.vector.tensor_add(gsum[:], gsum[:], gmax1[:])

                # pick top-4 groups; additive mask 0 / -BIG
                g8 = wpool.tile([P, 8], F32, tag="g8")
                nc.vector.max(g8[:], gsum[:])
                gneg = wpool.tile([P, N_GROUP], F32, tag="gneg")
                nc.vector.tensor_scalar(
                    gneg[:], gsum[:], g8[:, TOPK_GROUP - 1:TOPK_GROUP], NEG,
                    op0=OP.is_lt, op1=OP.mult,
                )

                # masked biased scores; top-8 experts
                tmp = wpool.tile([P, E], F32, tag="tmp")
                tmp3 = tmp[:].rearrange("p (g e) -> p g e", g=N_GROUP)
                nc.vector.tensor_tensor(
                    tmp3, sfc3, gneg[:, :, None].to_broadcast([P, N_GROUP, EG]),
                    op=OP.add,
                )
                max8 = wpool.tile([P, 8], F32, tag="max8")
                nc.vector.max(max8[:], tmp[:])
                idx8 = idx_all[:, j, :]
                nc.vector.max_index(idx8, max8[:], tmp[:])

                # selected-set mask via match_replace diff; sm = scores
                # where selected else ~NEG (Pool rejects comparison ALU
                # ops, so these stay on DVE)
                zap = wpool.tile([P, E], F32, tag="zap")
                nc.vector.match_replace(
                    zap[:], in_to_replace=max8[:], in_values=tmp[:],
                    imm_value=NEG,
                )
                q = wpool.tile([P, E], F32, tag="q")
                nc.vector.scalar_tensor_tensor(
                    q[:], tmp[:], 1.0, zap[:], op0=OP.mult, op1=OP.is_equal,
                )
                sm = wpool.tile([P, E], F32, tag="sm")
                nc.vector.scalar_tensor_tensor(
                    sm[:], q[:], NEG, scores[:], op0=OP.mult, op1=OP.add,
                )

                # unbiased scores of the selected 8, sorted by score
                smax8 = wpool.tile([P, 8], F32, tag="smax8")
                nc.vector.max(smax8[:], sm[:])
                sidx8 = wpool.tile([P, 8], U32, tag="sidx8")
                nc.vector.max_index(sidx8[:], smax8[:], sm[:])

                # reorder scores to biased-rank order:
                # w8[k] = sum_j smax8[j] * (sidx8[j] == idx8[k])
                idxf = wpool.tile([P, 8], F32, tag="idxf")
                nc.vector.tensor_copy(idxf[:], idx8)
                sidxf = wpool.tile([P, 8], F32, tag="sidxf")
                nc.vector.tensor_copy(sidxf[:], sidx8[:])
                eq = wpool.tile([P, 8, 8], F32, tag="eq")
                nc.vector.tensor_tensor(
                    eq[:],
                    idxf[:, :, None].to_broadcast([P, 8, 8]),
                    sidxf[:, None, :].to_broadcast([P, 8, 8]),
                    op=OP.is_equal,
                )
                wprod = wpool.tile([P, 8, 8], F32, tag="wprod")
                nc.vector.tensor_tensor(
                    wprod[:], eq[:], smax8[:, None, :].to_broadcast([P, 8, 8]),
                    op=OP.mult,
                )
                w8 = wpool.tile([P, 8], F32, tag="w8")
                nc.vector.reduce_sum(w8[:], wprod[:], axis=mybir.AxisListType.X)

                # normalize: w = 2.5 * w / sum(w)  (the reference's +1e-20
                # is a no-op at fp32 for den >= 8*sigmoid(min logit))
                den = wpool.tile([P, 1], F32, tag="den")
                nc.vector.reduce_sum(den[:], w8[:], axis=mybir.AxisListType.X)
                rden = wpool.tile([P, 1], F32, tag="rden")
                nc.vector.reciprocal(rden[:], den[:])
                nc.vector.tensor_scalar(
                    w_all[:, j, :], w8[:], rden[:], ROUTED_SCALING,
                    op0=OP.mult, op1=OP.mult,
                )

            for j in range(TOK_TILES):
                xh = xhpool.tile([P, KC, P], F16, tag="xh")
                xh_dma = nc.sync.dma_start(xh[:], xh16_4[:, j])
                _reserve(nc, nc.sync, xh_dma, 3, prev=prev_dma)
                xl = xl8pool.tile([P, KC, P], F8, tag="xl")
                xl_dma = nc.sync.dma_start(xl[:], xl8_4[:, j])
                _reserve(nc, nc.sync, xl_dma, 3, prev=xh_dma)
                x8 = xh8pool.tile([P, KC, P], F8, tag="x8")
                x8_dma = nc.sync.dma_start(x8[:], xh8_4[:, j])
                _reserve(nc, nc.sync, x8_dma, 3, prev=xl_dma)
                prev_dma = x8_dma
                if j == 0:
                    wc_dma = nc.sync.dma_start(wc[:], wc8_3)
                    _reserve(nc, nc.sync, wc_dma, 2, prev=prev_dma)
                    b_dma = nc.sync.dma_start(bsb[:], biasb.ap())
                    _reserve(nc, nc.sync, b_dma, 2, prev=wc_dma)
                    prev_dma = b_dma

                ps = pspool.tile([P, E], F32, tag="ps")
                psc = pspool.tile([P, E], F32, tag="psc")
                # The fused matmul (self-loading LDWEIGHTS) only has budget
                # for ONE semaphore wait in walrus codegen, but the
                # tile-leading matmul needs the x-DMA sems plus the
                # psum-slot-release sems. Emit a PE NoOp that declares those
                # data deps (1-elem APs, registered via annotate_deps) so
                # Tile's per-engine clock absorbs all waits there; the
                # matmuls then follow wait-free in PE program order. Tile
                # strips APs from InstNoOp at lowering, so walrus only
                # sees a plain NOP.
                gate = nc.tensor.nop(nofuse=True)
                gate.ins.ins = [nc.tensor.lower_ap(xh[0:1, 0, 0:1])]
                gate.ins.outs = [nc.tensor.lower_ap(ps[0:1, 0:1])]
                annotate_deps(tc.dep_state, gate.ins, tc.shadow_memory,
                              tc._rust_ctx, nc.inst_map)
                _reserve(nc, nc.tensor, gate, 4, prev=prev_mm)
                # Main fp16 pass (one contiguous accumulation group —
                # interleaving two groups miscompiles; the PE's 64-deep
                # reorder window still lets the correction group's
                # LDWEIGHTS creep into this stream).
                for i in range(KC):
                    nc.tensor.matmul(
                        ps[:], lhsT=xh[:, i, :], rhs=wh[:, i, :],
                        start=(i == 0), stop=(i == KC - 1),
                    )
                # Correction fp8 DoubleRow pass: pair i contracts chunks
                # (2i, 2i+1); pairs 0..15 are the xl-block, 16..31 the
                # xh-block.
                gate2 = nc.tensor.nop(nofuse=True)
                gate2.ins.ins = [
                    nc.tensor.lower_ap(xl[0:1, 0, 0:1]),
                    nc.tensor.lower_ap(x8[0:1, 0, 0:1]),
                ]
                gate2.ins.outs = [nc.tensor.lower_ap(psc[0:1, 0:1])]
                annotate_deps(tc.dep_state, gate2.ins, tc.shadow_memory,
                              tc._rust_ctx, nc.inst_map)
                _reserve(nc, nc.tensor, gate2, 4, prev=gate)
                for i in range(KC):
                    if i < KC // 2:
                        clhs, coff = xl, 2 * i
                    else:
                        clhs, coff = x8, 2 * (i - KC // 2)
                    mm = nc.tensor.matmul(
                        psc[:],
                        lhsT=clhs[:, coff:coff + 2, :],
                        rhs=wc[:, 2 * i:2 * i + 2, :],
                        start=(i == 0), stop=(i == KC - 1),
                        perf_mode=mybir.MatmulPerfMode.DoubleRow,
                    )
                prev_mm = mm

                # ---- logits = ps + 2^-16 * psc, sigmoid ----
                # Engines may read only ONE input from PSUM per instruction
                # (and GpSimd none), so: ACT scaled-copies psc to SBUF
                # (Copy bypasses the activation table, so no table thrash
                # with Sigmoid), then DVE adds ps (PSUM) + cb (SBUF).
                cb = wpool.tile([P, E], F32, tag="cb")
                cp = nc.scalar.activation(cb[:], psc[:], AF.Copy,
                                          scale=CORR_SCALE)
                _reserve(nc, nc.scalar, cp, 3, prev=prev_sig)
                u = wpool.tile([P, E], F32, tag="u")
                stt = nc.vector.tensor_add(u[:], ps[:], cb[:])
                _reserve(nc, nc.vector, stt, 3, prev=prev_dve)
                prev_dve = stt
                scores = wpool.tile([P, E], F32, tag="scores")
                sig = nc.scalar.activation(scores[:], u[:], AF.Sigmoid)
                _reserve(nc, nc.scalar, sig, 2, prev=cp)
                prev_sig = sig
                sfc = wpool.tile([P, E], F32, tag="sfc")
                badd = nc.gpsimd.tensor_add(sfc[:], scores[:], bsb[:])
                _reserve(nc, nc.gpsimd, badd, 2, prev=prev_gp)
                prev_gp = badd

                # Software-pipeline the top-k: tile j's post-sigmoid DVE
                # chain is emitted AFTER tile j+1's PSUM combine, so the
                # DVE's in-order stream never idles waiting for the
                # ACT-sigmoid -> GpSimd-bias round trip of the same tile.
                if pending is not None:
                    emit_topk(*pending)
                pending = (j, scores, sfc)
            emit_topk(*pending)

            d1 = nc.sync.dma_start(idx3, idx_all[:])
            _reserve(nc, nc.sync, d1, 2, prev=prev_dma)
            d2 = nc.sync.dma_start(w3, w_all[:])
            _reserve(nc, nc.sync, d2, 2, prev=d1)

    _legalize_waits(nc)
    return nc


class _Runner:
    """Compile-once SPMD runner (mirrors bass2jax.run_bass_via_pjrt's
    multi-core path, but holds the jitted fn so repeated calls don't
    re-trace/re-jit; inputs can stay resident on device for timing)."""

    def __init__(self, nc):
        import jax
        from jax.experimental.shard_map import shard_map
        from jax.sharding import Mesh, NamedSharding, PartitionSpec

        from concourse import bass2jax

        bass2jax.install_neuronx_cc_hook()
        self._jax = jax
        self.nc = nc

        partition_name = (
            nc.partition_id_tensor.name if nc.partition_id_tensor else None
        )
        in_names, out_names, out_avals, zero_outs = [], [], [], []
        for alloc in nc.m.functions[0].allocations:
            if not isinstance(alloc, mybir.MemoryLocationSet):
                continue
            name = alloc.memorylocations[0].name
            if alloc.kind == "ExternalInput":
                if name != partition_name:
                    in_names.append(name)
            elif alloc.kind == "ExternalOutput":
                shape = tuple(alloc.tensor_shape)
                dtype = mybir.dt.np(alloc.dtype)
                out_names.append(name)
                out_avals.append(jax.core.ShapedArray(shape, dtype))
                zero_outs.append(np.zeros(shape, dtype))
        self.in_names = list(in_names)
        self.out_names = out_names
        self.out_avals = out_avals
        self.zero_outs = zero_outs
        n_params = len(in_names)
        self.n_params = n_params

        all_names = in_names + out_names
        if partition_name is not None:
            all_names.append(partition_name)

        def _body(*args):
            operands = list(args)
            if partition_name is not None:
                operands.append(bass2jax.partition_id_tensor())
            outs = bass2jax._bass_exec_p.bind(
                *operands,
                out_avals=tuple(out_avals),
                in_names=tuple(all_names),
                out_names=tuple(out_names),
                lowering_input_output_aliases=(),
                sim_require_finite=True,
                sim_require_nnan=True,
                nc=nc,
            )
            return tuple(outs)

        devices = jax.devices()[:N_CORES]
        assert len(devices) == N_CORES
        self.mesh = Mesh(np.asarray(devices), ("core",))
        n_outs = len(out_names)
        in_specs = (PartitionSpec("core"),) * (n_params + n_outs)
        out_specs = (PartitionSpec("core"),) * n_outs
        donate = tuple(range(n_params, n_params + n_outs))
        self._fn = jax.jit(
            shard_map(
                _body, mesh=self.mesh, in_specs=in_specs, out_specs=out_specs,
                check_rep=False,
            ),
            donate_argnums=donate,
            keep_unused=True,
        )
        self._sharding = NamedSharding(self.mesh, PartitionSpec("core"))

    def put_inputs(self, in_maps):
        """Concat per-core inputs on axis 0 and move to device once."""
        concat = [
            np.concatenate([np.asarray(m[name]) for m in in_maps], axis=0)
            for name in self.in_names
        ]
        return [self._jax.device_put(a, self._sharding) for a in concat]

    def _zeros(self):
        return [
            np.zeros((N_CORES * z.shape[0], *z.shape[1:]), z.dtype)
            for z in self.zero_outs
        ]

    def execute(self, dev_inputs):
        outs = self._fn(*dev_inputs, *self._zeros())
        self._jax.block_until_ready(outs)
        return outs

    def run(self, in_maps):
        dev_inputs = self.put_inputs(in_maps)
        out_arrs = self.execute(dev_inputs)
        return [
            {
                name: np.asarray(out_arrs[i]).reshape(
                    N_CORES, *self.out_avals[i].shape
                )[c]
                for i, name in enumerate(self.out_names)
            }
            for c in range(N_CORES)
        ]


_RUNNER_CACHE = {}


def _get_runner():
    if "nc" not in _RUNNER_CACHE:
        _RUNNER_CACHE["nc"] = _Runner(build_nc())
    return _RUNNER_CACHE["nc"]


def make_in_maps(hidden_states, weight, e_score_correction_bias):
    x = np.ascontiguousarray(np.asarray(hidden_states), dtype=np.float32)
    x = x.reshape(T_FULL, H)
    w = np.asarray(weight, dtype=np.float32)
    b = np.asarray(e_score_correction_bias, dtype=np.float32)

    # Split: v = hi + lo with hi = fp16(v), lo = v - hi (exact in f32).
    x_hi = x.astype(F16_NP)
    x_lo = x - x_hi.astype(np.float32)
    w_hi = w.astype(F16_NP)
    w_lo = w - w_hi.astype(np.float32)

    def pack_x(xs):
        # [2048, 4096] -> [j, t, c, p] -> [j, p, c, t] -> [(j p), (c t)]
        return np.ascontiguousarray(
            xs.reshape(TOK_TILES, P, KC, P).transpose(0, 3, 2, 1)
        ).reshape(TOK_TILES * P, KC * P)

    def pack_w(ws):
        # [256, 4096] f32 -> [c, p, e] -> [p, c, e]
        return np.ascontiguousarray(
            ws.T.reshape(KC, P, E).transpose(1, 0, 2)
        )

    wh16 = pack_w(w_hi.astype(np.float32)).astype(F16_NP).reshape(P, KC * E)
    wc8 = np.concatenate(
        [pack_w(w_hi.astype(np.float32) * S2), pack_w(w_lo * S4)], axis=1
    ).astype(F8_NP).reshape(P, 2 * KC * E)
    biasb = np.ascontiguousarray(np.broadcast_to(b, (P, E)))

    in_maps = []
    for i in range(N_CORES):
        sl = slice(i * T_CORE, (i + 1) * T_CORE)
        in_maps.append({
            "xh16": pack_x(x_hi[sl].astype(np.float32)).astype(F16_NP),
            "xl8": pack_x(x_lo[sl] * S1).astype(F8_NP),
            "xh8": pack_x(x_hi[sl].astype(np.float32) * S3).astype(F8_NP),
            "wh16": wh16,
            "wc8": wc8,
            "biasb": biasb,
        })
    return in_maps


def kernel(hidden_states, weight, e_score_correction_bias):
    runner = _get_runner()
    results = runner.run(
        make_in_maps(hidden_states, weight, e_score_correction_bias)
    )
    topk_idx = np.concatenate(
        [r["idx_out"].astype(np.int32) for r in results], axis=0
    )
    topk_weight = np.concatenate([r["w_out"] for r in results], axis=0)
    return topk_idx, topk_weight
